# revision 13
# baseline (speedup 1.0000x reference)
"""BiAttention TRN2 kernel: data-parallel over batch across 8 NeuronCores.

Self-contained: hardcodes B=32, Tc=2048, Tq=256, D=256, 8 cores, 4 batches/core.
Raw-bass software-pipelined kernel; f32r matmuls; exact power-of-two mask trick.
"""
import numpy as np

import concourse.bass as bass
from concourse import mybir
from concourse.bass_utils import run_bass_kernel_spmd

F32 = mybir.dt.float32
F32R = mybir.dt.float32r
BF16 = mybir.dt.bfloat16
Exp = mybir.ActivationFunctionType.Exp
AX = mybir.AxisListType
OP = mybir.AluOpType

B, TC, TQ, D = 32, 2048, 256, 256
NCORES = 8
NB = B // NCORES          # batches per core = 4
NBLK = TC // 128          # c-blocks per batch = 16
NEG = -(2.0 ** 96)
SQ = 2.0 ** 48


def build_program():
    nc = bass.Bass()
    c_d = nc.declare_dram_parameter("c", [NB, TC, D], F32, isOutput=False)
    q_d = nc.declare_dram_parameter("q", [NB, TQ, D], F32, isOutput=False)
    mc_d = nc.declare_dram_parameter("mcf", [NB, 2, TC], F32, isOutput=False)
    mq_d = nc.declare_dram_parameter("mqf", [NB, 2, TQ], F32, isOutput=False)
    id_d = nc.declare_dram_parameter("ident", [128, 128], F32, isOutput=False)
    onew_d = nc.declare_dram_parameter("onesw", [128, 256], F32, isOutput=False)
    c100_d = nc.declare_dram_parameter("c100", [128, 1], F32, isOutput=False)

    o_d = nc.declare_dram_parameter("o", [NB, TC, D], F32, isOutput=True)
    qc_d = nc.declare_dram_parameter("qc", [NB, TQ], F32, isOutput=True)

    from contextlib import ExitStack
    es = ExitStack()
    _ctr = [0]

    def sb(shape, dt, name=None):
        _ctr[0] += 1
        return es.enter_context(nc.sbuf_tensor(name or f"sb{_ctr[0]}", shape, dt))

    def ps(shape, dt, name=None):
        _ctr[0] += 1
        return es.enter_context(nc.psum_tensor(name or f"ps{_ctr[0]}", shape, dt))

    def sem(name):
        return es.enter_context(nc.semaphore(name))

    # ---- SBUF ----
    cb = [sb([128, NBLK, D], F32R) for _ in range(2)]      # C natural (f32r), per-batch parity
    qn = [sb([128, 2, D], F32R) for _ in range(2)]          # Q natural [q%128, qchunk, d]
    qtr = [sb([128, 2, TQ], F32R) for _ in range(2)]        # Q^T [d%128, dchunk, q]
    mcs = [sb([2, TC], F32R) for _ in range(2)]             # mask lhsT features
    mqs = [sb([2, TQ], F32R) for _ in range(2)]             # mask rhs features
    ident = sb([128, 128], F32R)
    ones_w = sb([128, 256], F32R)                           # all-ones (total-sum rhs)
    c100 = sb([128, 1], F32)                                # bias constant -100
    ctr = [sb([128, 2, 128], F32R) for _ in range(2)]       # C^T chunks, block parity
    ptr = [sb([128, 2, 128], BF16) for _ in range(2)]       # P^T chunks (bf16), block parity
    p_sb = [sb([128, TQ], BF16) for _ in range(2)]          # exp(S-m) (bf16), block parity
    qn_b = [sb([128, 2, D], BF16) for _ in range(2)]        # Q natural bf16 (mm2 rhs)
    ident_b = sb([128, 128], BF16)
    o_all = [sb([128, NBLK, D], F32) for _ in range(2)]     # output batch buffer
    NM = [sb([128, NBLK], F32) for _ in range(2)]           # -rowmax per block column
    SS = [sb([128, NBLK], F32) for _ in range(2)]           # rowsum per block column
    RS = [sb([128, NBLK], F32) for _ in range(2)]           # 1/rowsum
    E_all = sb([128, NBLK], F32R)                           # exp(m - 100) for q2c
    esum = sb([128, 1], F32)
    esum_r = sb([128, 1], F32R)
    t_sb = sb([1, 1], F32)
    rtot = sb([1, 1], F32)
    qc_sb = [sb([1, TQ], F32) for _ in range(2)]

    # ---- PSUM (bank-granular allocator: 8 banks total) ----
    pJ = [ps([128, 512], F32R) for _ in range(2)]   # CT at [:,0:256], PT at [:,256:512]
    pS = [ps([128, 256], F32) for _ in range(2)]
    pO = [ps([128, 256], F32) for _ in range(2)]
    pQT = ps([128, 2, 256], F32R)                   # batch prep QT; tail nmin-transpose at [0:1,0,0:128]
    # pM regions: pQC=[0:1,0:256], pTot=[0:1,256:512]
    pM = ps([128, 512], F32)

    sems = {}
    for name in ("s_cin", "s_out", "s_qc", "pe_ct", "pe_qt", "pe_s", "pe_pt",
                 "pe_o", "pt_", "dve_ctr", "dve_qtr", "dve_nm", "dve_rs",
                 "dve_ptr", "dt", "act_p", "act_o", "at", "s_misc"):
        sems[name] = sem(name)
    s_cin = sems["s_cin"]; s_out = sems["s_out"]; s_qc = sems["s_qc"]
    pe_ct = sems["pe_ct"]; pe_qt = sems["pe_qt"]; pe_s = sems["pe_s"]
    pe_pt = sems["pe_pt"]; pe_o = sems["pe_o"]; pt_ = sems["pt_"]
    dve_ctr = sems["dve_ctr"]; dve_qtr = sems["dve_qtr"]; dve_nm = sems["dve_nm"]
    dve_rs = sems["dve_rs"]; dve_ptr = sems["dve_ptr"]; dt = sems["dt"]
    act_p = sems["act_p"]; act_o = sems["act_o"]; at = sems["at"]
    s_misc = sems["s_misc"]

    blk = es.enter_context(nc.Block())
    with blk:
        # ---------------- GPSIMD: input cast-DMAs ----------------
        @blk.gpsimd
        def _(g):
            for b in range(NB):
                if b >= 2:
                    g.wait_ge(pt_, b - 1)
                if b >= 1:
                    # all previously issued input DMAs must have completed so
                    # cumulative thresholds are meaningful (unordered DMA completion)
                    g.wait_ge(s_cin, 64 * b + 48)
                g.dma_start(cb[b % 2][:], c_d[b].rearrange("(i p) d -> p i d", p=128)).then_inc(s_cin, 16)
                g.dma_start(qn[b % 2][:], q_d[b].rearrange("(a p) d -> p a d", p=128)).then_inc(s_cin, 16)
                g.dma_start(mcs[b % 2][:], mc_d[b]).then_inc(s_cin, 16)
                g.dma_start(mqs[b % 2][:], mq_d[b]).then_inc(s_cin, 16)
                if b == 0:
                    g.dma_start(ident[:], id_d[:]).then_inc(s_cin, 16)
                    g.dma_start(ones_w[:], onew_d[:]).then_inc(s_cin, 16)
                    g.dma_start(c100[:], c100_d[:]).then_inc(s_cin, 16)

        def cin_thresh(b):
            return 64 * (b + 1) + 48

        # ---------------- PE ----------------
        @blk.tensor
        def _(t):
            def ct_tr(n):
                b, i = divmod(n, NBLK)
                k = n % 2
                if i == 0:
                    t.wait_ge(s_cin, cin_thresh(b))
                tr0 = t.transpose(pJ[k][:, 0:128], cb[b % 2][:, i, 0:128], ident[:])
                if n >= 1:
                    tr0._wait_ge(dve_ptr, n - 1)
                t.transpose(pJ[k][:, 128:256], cb[b % 2][:, i, 128:256], ident[:]).then_inc(pe_ct, 1)

            def sim(n):
                b, i = divmod(n, NBLK)
                k = n % 2
                t.wait_ge(dve_ctr, n + 1)
                if i == 0:
                    t.wait_ge(dve_qtr, b + 1)
                mm0 = t.matmul(pS[k][:], mcs[b % 2][:, i * 128:(i + 1) * 128], mqs[b % 2][:],
                               start=True, stop=False)
                if n >= 2:
                    mm0._wait_ge(act_p, n - 1)
                t.matmul(pS[k][:], ctr[k][:, 0], qtr[b % 2][:, 0], start=False, stop=False)
                t.matmul(pS[k][:], ctr[k][:, 1], qtr[b % 2][:, 1], start=False, stop=True).then_inc(pe_s, 1)

            def pt_tr(n):
                k = n % 2
                if n == 0:
                    t.wait_ge(s_misc, 1)    # ident_b ready
                if n >= 2:
                    t.wait_ge(dve_ptr, n - 1)
                ptb = pJ[k][:].bitcast(BF16)
                tr0 = t.transpose(ptb[:, 512:640], p_sb[k][:, 0:128], ident_b[:])
                tr0._wait_ge(act_p, n + 1)
                t.transpose(ptb[:, 640:768], p_sb[k][:, 128:256], ident_b[:]).then_inc(pe_pt, 1)

            def mm2(n):
                b, i = divmod(n, NBLK)
                k = n % 2
                if n >= 2:
                    t.wait_ge(act_o, n - 1)
                mm0 = t.matmul(pO[k][:], ptr[k][:, 0], qn_b[b % 2][:, 0], start=True, stop=False)
                mm0._wait_ge(dve_ptr, n + 1)
                t.matmul(pO[k][:], ptr[k][:, 1], qn_b[b % 2][:, 1], start=False, stop=True).then_inc(pe_o, 1)

            def qt_prep(b):
                t.wait_ge(s_cin, cin_thresh(b))
                if b >= 1:
                    t.wait_ge(dve_qtr, b)   # prev batch qtr copy done (pQT bank free)
                last = None
                for qa in range(2):
                    for kk in range(2):
                        last = t.transpose(
                            pQT[:, kk, qa * 128:(qa + 1) * 128],
                            qn[b % 2][:, qa, kk * 128:(kk + 1) * 128],
                            ident[:],
                        )
                last.then_inc(pe_qt, 1)

            def tail(b):
                # C: q2c matmuls + total sum (constant-shift exp, no global max)
                t.wait_ge(dt, 2 * b + 1)      # esum_r ready
                t.wait_ge(at, 2 * b + 1)      # E_all ready
                if b >= 1:
                    t.wait_ge(at, 2 * b)      # T2(b-1) done reading pM
                for i in range(NBLK):
                    t.matmul(pM[0:1, 0:256], E_all[:, i:i + 1], cb[b % 2][:, i, :],
                             start=(i == 0), stop=(i == NBLK - 1))
                t.matmul(pM[0:1, 256:512], esum_r[:], ones_w[:], start=True,
                         stop=True).then_inc(pt_, 1)

            for b in range(NB):
                qt_prep(b)
                for slot in range(NBLK + 3):
                    i = slot - 2
                    if 0 <= i <= NBLK - 1:
                        pt_tr(16 * b + i)
                    i = slot - 3
                    if 0 <= i <= NBLK - 1:
                        mm2(16 * b + i)
                    i = slot
                    if 0 <= i <= NBLK - 1:
                        ct_tr(16 * b + i)
                    i = slot - 1
                    if 0 <= i <= NBLK - 1:
                        sim(16 * b + i)
                tail(b)

        # ---------------- DVE ----------------
        @blk.vector
        def _(v):
            def qtr_copy(b):
                if b == 0:
                    v.wait_ge(s_cin, cin_thresh(0))
                    v.tensor_copy(ident_b[:], ident[:]).then_inc(s_misc, 1)
                v.wait_ge(pe_qt, b + 1)
                if b >= 2:
                    v.wait_ge(pe_o, 16 * (b - 1))   # qn_b WAR (implies pe_s too)
                v.tensor_copy(qn_b[b % 2][:], qn[b % 2][:])
                v.tensor_copy(qtr[b % 2][:], pQT[:]).then_inc(dve_qtr, 1)

            def ctr_copy(n):
                k = n % 2
                if n >= 2:
                    v.wait_ge(pe_s, n - 1)
                cp = v.tensor_copy(ctr[k][:], pJ[k][:, 0:256])
                cp._wait_ge(pe_ct, n + 1)
                cp.then_inc(dve_ctr, 1)

            def nm(n):
                b, i = divmod(n, NBLK)
                k = n % 2
                if i == 0 and b >= 2:
                    v.wait_ge(at, 2 * (b - 2) + 1)   # tail(b-2) E-exp read NM buffer
                rd = v.tensor_reduce(NM[b % 2][:, i:i + 1], pS[k][:], AX.X, OP.max,
                                     negate=True)
                rd._wait_ge(pe_s, n + 1)
                rd.then_inc(dve_nm, 1)

            def ptr_copy(n):
                k = n % 2
                if n >= 2:
                    v.wait_ge(pe_o, n - 1)
                cp = v.tensor_copy(ptr[k][:], pJ[k][:].bitcast(BF16)[:, 512:768])
                cp._wait_ge(pe_pt, n + 1)
                cp.then_inc(dve_ptr, 1)

            def recip(n):
                b, i = divmod(n, NBLK)
                if i == 0 and b >= 2:
                    v.wait_ge(act_o, 16 * (b - 1))   # RS WAR vs out-copy of b-2
                rc = v.reciprocal(RS[b % 2][:, i:i + 1], SS[b % 2][:, i:i + 1])
                rc._wait_ge(act_p, n + 1)
                rc.then_inc(dve_rs, 1)

            def tail(b):
                # X1: esum -> f32r
                v.wait_ge(at, 2 * b + 1)
                v.tensor_copy(esum_r[:], esum[:]).then_inc(dt, 1)
                # X2: total -> reciprocal
                v.wait_ge(pt_, b + 1)
                if b >= 1:
                    v.wait_ge(at, 2 * b)   # T2(b-1) done with rtot
                v.tensor_copy(t_sb[:], pM[0:1, 256:257])
                v.drain()
                v.reciprocal(rtot[:], t_sb[:]).then_inc(dt, 1)

            for b in range(NB):
                qtr_copy(b)
                for slot in range(NBLK + 3):
                    i = slot - 2
                    if 0 <= i <= NBLK - 1:
                        ptr_copy(16 * b + i)
                        recip(16 * b + i)
                    i = slot
                    if 0 <= i <= NBLK - 1:
                        ctr_copy(16 * b + i)
                    i = slot - 1
                    if 0 <= i <= NBLK - 1:
                        nm(16 * b + i)
                tail(b)

        # ---------------- ACT ----------------
        @blk.scalar
        def _(s):
            def ex(n):
                b, i = divmod(n, NBLK)
                k = n % 2
                if n >= 2:
                    s.wait_ge(pe_pt, n - 1)
                if i == 0 and b >= 2:
                    s.wait_ge(dve_rs, 16 * (b - 1))  # SS WAR vs recip of b-2
                ac = s.activation(p_sb[k][:], pS[k][:], Exp,
                                  bias=NM[b % 2][:, i:i + 1],
                                  accum_out=SS[b % 2][:, i:i + 1])
                ac._wait_ge(dve_nm, n + 1)
                ac.then_inc(act_p, 1)

            def outcp(n):
                b, i = divmod(n, NBLK)
                k = n % 2
                s.wait_ge(dve_rs, n + 1)
                if i == 0 and b >= 2:
                    s.wait_ge(s_out, 16 * (b - 1))
                oc = s.mul(o_all[b % 2][:, i, :], pO[k][:], RS[b % 2][:, i:i + 1])
                oc._wait_ge(pe_o, n + 1)
                oc.then_inc(act_o, 1)

            def tail(b):
                # T1: E = exp(-NM - 100), accum esum
                s.wait_ge(dve_nm, 16 * (b + 1))
                if b >= 1:
                    s.wait_ge(pt_, b)        # E_all/esum WAR vs tail C of b-1
                s.activation(E_all[:], NM[b % 2][:], Exp, bias=c100[:], scale=-1.0,
                             accum_out=esum[:]).then_inc(at, 1)
                # T2: qc = pQC * rtot
                s.wait_ge(dt, 2 * b + 2)
                s.wait_ge(pt_, b + 1)
                if b >= 2:
                    s.wait_ge(s_qc, 16 * (b - 1))
                s.mul(qc_sb[b % 2][:], pM[0:1, 0:256], rtot[:]).then_inc(at, 1)

            for b in range(NB):
                for slot in range(NBLK + 3):
                    i = slot - 1
                    if 0 <= i <= NBLK - 1:
                        ex(16 * b + i)
                    i = slot - 3
                    if 0 <= i <= NBLK - 1:
                        outcp(16 * b + i)
                tail(b)

        # ---------------- SYNC: output DMAs ----------------
        @blk.sync
        def _(sy):
            for b in range(NB):
                sy.wait_ge(act_o, 16 * (b + 1))
                if b >= 1:
                    sy.wait_ge(s_out, 16 * b)
                sy.dma_start(o_d[b].rearrange("(i p) d -> p i d", p=128),
                             o_all[b % 2][:]).then_inc(s_out, 16)
                sy.wait_ge(at, 2 * b + 2)
                if b >= 1:
                    sy.wait_ge(s_qc, 16 * b)
                sy.dma_start(qc_d[b:b + 1, :], qc_sb[b % 2][:]).then_inc(s_qc, 16)

    return nc, es


_CACHE = {}


def _get_program():
    if "nc" not in _CACHE:
        nc, es = build_program()
        _CACHE["nc"] = nc
        _CACHE["es"] = es
    return _CACHE["nc"]


def kernel(context_repr, question_repr, context_len, question_len):
    context_repr = np.ascontiguousarray(np.asarray(context_repr, np.float32))
    question_repr = np.ascontiguousarray(np.asarray(question_repr, np.float32))
    context_len = np.asarray(context_len, np.int32)
    question_len = np.asarray(question_len, np.int32)

    cm = (np.arange(TC)[None, :] < context_len[:, None]).astype(np.float32)  # [B,Tc]
    qm = (np.arange(TQ)[None, :] < question_len[:, None]).astype(np.float32)  # [B,Tq]
    mcf = np.stack([SQ * cm, np.ones_like(cm)], axis=1)                      # [B,2,Tc]
    mqf = np.stack([SQ * qm, np.full_like(qm, NEG)], axis=1)                 # [B,2,Tq]
    ident = np.eye(128, dtype=np.float32)
    onesw = np.ones((128, 256), np.float32)
    c100 = np.full((128, 1), -100.0, np.float32)

    nc = _get_program()
    in_maps = []
    for core in range(NCORES):
        sl = slice(core * NB, (core + 1) * NB)
        in_maps.append({
            "c": context_repr[sl],
            "q": question_repr[sl],
            "mcf": np.ascontiguousarray(mcf[sl]),
            "mqf": np.ascontiguousarray(mqf[sl]),
            "ident": ident,
            "onesw": onesw,
            "c100": c100,
        })

    res = run_bass_kernel_spmd(nc, in_maps, list(range(NCORES)))
    out1 = np.concatenate([np.asarray(r["o"]).reshape(NB, TC, D) for r in res.results], axis=0)
    q2c = np.concatenate([np.asarray(r["qc"]).reshape(NB, TQ) for r in res.results], axis=0)
    out2 = np.ascontiguousarray(np.broadcast_to(q2c[:, None, :], (B, TC, D)))
    return out1, out2


# revision 15
# speedup vs baseline: 1.2418x; 1.2418x over previous
"""BiAttention TRN2 kernel: data-parallel over batch across 8 NeuronCores.

Self-contained: hardcodes B=32, Tc=2048, Tq=256, D=256, 8 cores, 4 batches/core.
Raw-bass software-pipelined kernel; f32r matmuls; exact power-of-two mask trick.
"""
import numpy as np

import concourse.bass as bass
from concourse import mybir
from concourse.bass_utils import run_bass_kernel_spmd

F32 = mybir.dt.float32
F32R = mybir.dt.float32r
BF16 = mybir.dt.bfloat16
Exp = mybir.ActivationFunctionType.Exp
AX = mybir.AxisListType
OP = mybir.AluOpType

B, TC, TQ, D = 32, 2048, 256, 256
NCORES = 8
NB = B // NCORES          # batches per core = 4
NBLK = TC // 128          # c-blocks per batch = 16
NEG = -(2.0 ** 96)
SQ = 2.0 ** 48


def build_program():
    nc = bass.Bass()
    c_d = nc.declare_dram_parameter("c", [NB, TC, D], F32, isOutput=False)
    q_d = nc.declare_dram_parameter("q", [NB, TQ, D], F32, isOutput=False)
    mc_d = nc.declare_dram_parameter("mcf", [NB, 2, TC], F32, isOutput=False)
    mq_d = nc.declare_dram_parameter("mqf", [NB, 2, TQ], F32, isOutput=False)
    id_d = nc.declare_dram_parameter("ident", [128, 128], F32, isOutput=False)
    onew_d = nc.declare_dram_parameter("onesw", [128, 256], F32, isOutput=False)
    c100_d = nc.declare_dram_parameter("c100", [128, 1], F32, isOutput=False)

    o_d = nc.declare_dram_parameter("o", [NB, TC, D], F32, isOutput=True)
    qc_d = nc.declare_dram_parameter("qc", [NB, TQ], F32, isOutput=True)

    from contextlib import ExitStack
    es = ExitStack()
    _ctr = [0]

    def sb(shape, dt, name=None):
        _ctr[0] += 1
        return es.enter_context(nc.sbuf_tensor(name or f"sb{_ctr[0]}", shape, dt))

    def ps(shape, dt, name=None):
        _ctr[0] += 1
        return es.enter_context(nc.psum_tensor(name or f"ps{_ctr[0]}", shape, dt))

    def sem(name):
        return es.enter_context(nc.semaphore(name))

    # ---- SBUF ----
    cb = [sb([128, NBLK, D], F32R) for _ in range(2)]      # C natural (f32r), per-batch parity
    qn = [sb([128, 2, D], F32R) for _ in range(2)]          # Q natural [q%128, qchunk, d]
    qtr = [sb([128, 2, TQ], F32R) for _ in range(2)]        # Q^T [d%128, dchunk, q]
    mcs = [sb([2, TC], F32R) for _ in range(2)]             # mask lhsT features
    mqs = [sb([2, TQ], F32R) for _ in range(2)]             # mask rhs features
    ident = sb([128, 128], F32R)
    ones_w = sb([128, 256], F32R)                           # all-ones (total-sum rhs)
    c100 = sb([128, 1], F32)                                # bias constant -100
    ctr = [sb([128, 2, 128], F32R) for _ in range(2)]       # C^T chunks, block parity
    ptr = [sb([128, 2, 128], BF16) for _ in range(2)]       # P^T chunks (bf16), block parity
    p_sb = [sb([128, TQ], BF16) for _ in range(4)]          # exp(S-m) (bf16), 4-deep
    qn_b = [sb([128, 2, D], BF16) for _ in range(2)]        # Q natural bf16 (mm2 rhs)
    ident_b = sb([128, 128], BF16)
    o_all = [sb([128, NBLK, D], F32) for _ in range(2)]     # output batch buffer
    NM = [sb([128, NBLK], F32) for _ in range(2)]           # -rowmax per block column
    SS = [sb([128, NBLK], F32) for _ in range(2)]           # rowsum per block column
    RS = [sb([128, NBLK], F32) for _ in range(2)]           # 1/rowsum
    E_all = sb([128, NBLK], F32R)                           # exp(m - 100) for q2c
    esum = sb([128, 1], F32)
    esum_r = sb([128, 1], F32R)
    t_sb = sb([1, 1], F32)
    rtot = sb([1, 1], F32)
    qc_sb = [sb([1, TQ], F32) for _ in range(2)]

    # ---- PSUM (bank-granular allocator: 8 banks total) ----
    pJ = [ps([128, 256], F32R) for _ in range(2)]   # C^T per block parity (1 bank each)
    pPT = [ps([128, 128], F32R) for _ in range(2)]  # P^T (bf16 via bitcast), 1 bank each
    pS = ps([128, 4, 256], F32)                     # sim quad (2 banks); QT prep borrows bank0 via f32r bitcast
    pO = ps([128, 2, 256], F32)                     # mm2 out, 2 slots in 1 bank
    # pM regions: pQC=[0:1,0:256], pTot=[0:1,256:512]
    pM = ps([128, 512], F32)

    sems = {}
    for name in ("s_cin", "s_out", "s_qc", "pe_ct", "pe_qt", "pe_s", "pe_pt",
                 "pe_o", "pt_", "dve_ctr", "dve_qtr", "dve_nm", "dve_rs",
                 "dve_ptr", "dt", "act_p", "act_o", "at", "s_misc"):
        sems[name] = sem(name)
    s_cin = sems["s_cin"]; s_out = sems["s_out"]; s_qc = sems["s_qc"]
    pe_ct = sems["pe_ct"]; pe_qt = sems["pe_qt"]; pe_s = sems["pe_s"]
    pe_pt = sems["pe_pt"]; pe_o = sems["pe_o"]; pt_ = sems["pt_"]
    dve_ctr = sems["dve_ctr"]; dve_qtr = sems["dve_qtr"]; dve_nm = sems["dve_nm"]
    dve_rs = sems["dve_rs"]; dve_ptr = sems["dve_ptr"]; dt = sems["dt"]
    act_p = sems["act_p"]; act_o = sems["act_o"]; at = sems["at"]
    s_misc = sems["s_misc"]

    blk = es.enter_context(nc.Block())
    with blk:
        # ---------------- GPSIMD: input cast-DMAs ----------------
        @blk.gpsimd
        def _(g):
            for b in range(NB):
                if b >= 2:
                    g.wait_ge(pt_, b - 1)
                if b >= 1:
                    # all previously issued input DMAs must have completed so
                    # cumulative thresholds are meaningful (unordered DMA completion)
                    g.wait_ge(s_cin, 64 * b + 48)
                g.dma_start(cb[b % 2][:], c_d[b].rearrange("(i p) d -> p i d", p=128)).then_inc(s_cin, 16)
                g.dma_start(qn[b % 2][:], q_d[b].rearrange("(a p) d -> p a d", p=128)).then_inc(s_cin, 16)
                g.dma_start(mcs[b % 2][:], mc_d[b]).then_inc(s_cin, 16)
                g.dma_start(mqs[b % 2][:], mq_d[b]).then_inc(s_cin, 16)
                if b == 0:
                    g.dma_start(ident[:], id_d[:]).then_inc(s_cin, 16)
                    g.dma_start(ones_w[:], onew_d[:]).then_inc(s_cin, 16)
                    g.dma_start(c100[:], c100_d[:]).then_inc(s_cin, 16)

        def cin_thresh(b):
            return 64 * (b + 1) + 48

        # ---------------- PE ----------------
        @blk.tensor
        def _(t):
            def ct_tr(n):
                b, i = divmod(n, NBLK)
                k = n % 2
                if i == 0:
                    t.wait_ge(s_cin, cin_thresh(b))
                tr0 = t.transpose(pJ[k][:, 0:128], cb[b % 2][:, i, 0:128], ident[:])
                if n >= 2:
                    tr0._wait_ge(dve_ctr, n - 1)
                t.transpose(pJ[k][:, 128:256], cb[b % 2][:, i, 128:256], ident[:]).then_inc(pe_ct, 1)

            def sim(n):
                b, i = divmod(n, NBLK)
                k = n % 2
                q = n % 4
                t.wait_ge(dve_ctr, n + 1)
                if i in (0, 1):
                    t.wait_ge(dve_qtr, b + 1)     # bank0 quarters held QT
                ap = n - 2 - (n % 2)              # exp of evicted/conflicting quarter done
                if ap >= 1:
                    t.wait_ge(act_p, ap)
                if n >= 4:
                    t.wait_ge(dve_nm, n // 4)     # quad reduce of prior group done
                mm0 = t.matmul(pS[:, q, :], mcs[b % 2][:, i * 128:(i + 1) * 128],
                               mqs[b % 2][:], start=True, stop=False)
                t.matmul(pS[:, q, :], ctr[k][:, 0], qtr[b % 2][:, 0], start=False, stop=False)
                t.matmul(pS[:, q, :], ctr[k][:, 1], qtr[b % 2][:, 1], start=False, stop=True).then_inc(pe_s, 1)

            def pt_tr(n):
                k = n % 2
                if n == 0:
                    t.wait_ge(s_misc, 1)    # ident_b ready
                if n >= 2:
                    t.wait_ge(dve_ptr, n - 1)
                ptb = pPT[k][:].bitcast(BF16)
                tr0 = t.transpose(ptb[:, 0:128], p_sb[n % 4][:, 0:128], ident_b[:])
                tr0._wait_ge(act_p, n + 1)
                t.transpose(ptb[:, 128:256], p_sb[n % 4][:, 128:256], ident_b[:]).then_inc(pe_pt, 1)

            def mm2(n):
                b, i = divmod(n, NBLK)
                k = n % 2
                if n >= 1:
                    t.wait_ge(act_o, n)       # pO single bank: outcp(n-1) done
                mm0 = t.matmul(pO[:, k, :], ptr[k][:, 0], qn_b[b % 2][:, 0], start=True, stop=False)
                mm0._wait_ge(dve_ptr, n + 1)
                t.matmul(pO[:, k, :], ptr[k][:, 1], qn_b[b % 2][:, 1], start=False, stop=True).then_inc(pe_o, 1)

            def qt_prep(b):
                t.wait_ge(s_cin, cin_thresh(b))
                if b >= 1:
                    t.wait_ge(dve_qtr, b)       # prev QT copy done
                    t.wait_ge(act_p, 16 * b)    # pS bank0 prior exps done
                    t.wait_ge(dve_nm, 4 * b)    # prior quad reads done
                psr = pS[:].bitcast(F32R)
                last = None
                for qa in range(2):
                    for kk in range(2):
                        last = t.transpose(
                            psr[:, kk, qa * 128:(qa + 1) * 128],
                            qn[b % 2][:, qa, kk * 128:(kk + 1) * 128],
                            ident[:],
                        )
                last.then_inc(pe_qt, 1)

            def tail(b):
                # C: q2c matmuls + total sum (constant-shift exp, no global max)
                t.wait_ge(dt, 2 * b + 1)      # esum_r ready
                t.wait_ge(at, 2 * b + 1)      # E_all ready
                if b >= 1:
                    t.wait_ge(at, 2 * b)      # T2(b-1) done reading pM
                for i in range(NBLK):
                    t.matmul(pM[0:1, 0:256], E_all[:, i:i + 1], cb[b % 2][:, i, :],
                             start=(i == 0), stop=(i == NBLK - 1))
                t.matmul(pM[0:1, 256:512], esum_r[:], ones_w[:], start=True,
                         stop=True).then_inc(pt_, 1)

            for b in range(NB):
                qt_prep(b)
                for slot in range(NBLK + 8):
                    i = slot - 5
                    if 0 <= i <= NBLK - 1:
                        pt_tr(16 * b + i)
                    i = slot - 6
                    if 0 <= i <= NBLK - 1:
                        mm2(16 * b + i)
                    i = slot
                    if 0 <= i <= NBLK - 1:
                        ct_tr(16 * b + i)
                    i = slot - 1
                    if 0 <= i <= NBLK - 1:
                        sim(16 * b + i)
                tail(b)

        # ---------------- DVE ----------------
        @blk.vector
        def _(v):
            def qtr_copy(b):
                if b == 0:
                    v.wait_ge(s_cin, cin_thresh(0))
                    v.tensor_copy(ident_b[:], ident[:]).then_inc(s_misc, 1)
                v.wait_ge(pe_qt, b + 1)
                if b >= 2:
                    v.wait_ge(pe_o, 16 * (b - 1))   # qn_b WAR (implies pe_s too)
                v.tensor_copy(qn_b[b % 2][:], qn[b % 2][:])
                v.tensor_copy(qtr[b % 2][:], pS[:].bitcast(F32R)[:, 0:2, :]).then_inc(dve_qtr, 1)

            def ctr_copy(n):
                k = n % 2
                if n >= 2:
                    v.wait_ge(pe_s, n - 1)
                cp = v.tensor_copy(ctr[k][:], pJ[k][:, 0:256])
                cp._wait_ge(pe_ct, n + 1)
                cp.then_inc(dve_ctr, 1)

            def nm_quad(b, qq):
                # one reduce for blocks 16b+4qq .. +3
                i4 = 4 * qq
                if qq == 0 and b >= 2:
                    v.wait_ge(at, 2 * (b - 2) + 1)   # tail(b-2) E-exp read NM buffer
                rd = v.tensor_reduce(NM[b % 2][:, i4:i4 + 4], pS[:], AX.X, OP.max,
                                     negate=True)
                rd._wait_ge(pe_s, 16 * b + 4 * qq + 4)
                rd.then_inc(dve_nm, 1)

            def ptr_copy(n):
                k = n % 2
                if n >= 2:
                    v.wait_ge(pe_o, n - 1)
                cp = v.tensor_copy(ptr[k][:], pPT[k][:].bitcast(BF16)[:, 0:256])
                cp._wait_ge(pe_pt, n + 1)
                cp.then_inc(dve_ptr, 1)

            def recip(n):
                b, i = divmod(n, NBLK)
                if i == 0 and b >= 2:
                    v.wait_ge(act_o, 16 * (b - 1))   # RS WAR vs out-copy of b-2
                rc = v.reciprocal(RS[b % 2][:, i:i + 1], SS[b % 2][:, i:i + 1])
                rc._wait_ge(act_p, n + 1)
                rc.then_inc(dve_rs, 1)

            def tail(b):
                # X1: esum -> f32r
                v.wait_ge(at, 2 * b + 1)
                v.tensor_copy(esum_r[:], esum[:]).then_inc(dt, 1)
                # X2: total -> reciprocal
                v.wait_ge(pt_, b + 1)
                if b >= 1:
                    v.wait_ge(at, 2 * b)   # T2(b-1) done with rtot
                v.tensor_copy(t_sb[:], pM[0:1, 256:257])
                v.drain()
                v.reciprocal(rtot[:], t_sb[:]).then_inc(dt, 1)

            for b in range(NB):
                qtr_copy(b)
                for slot in range(NBLK + 8):
                    i = slot - 6
                    if 0 <= i <= NBLK - 1:
                        ptr_copy(16 * b + i)
                    i = slot - 5
                    if 0 <= i <= NBLK - 1:
                        recip(16 * b + i)
                    i = slot
                    if 0 <= i <= NBLK - 1:
                        ctr_copy(16 * b + i)
                    if slot >= 4 and (slot - 4) % 4 == 0 and (slot - 4) // 4 <= 3:
                        nm_quad(b, (slot - 4) // 4)
                tail(b)

        # ---------------- ACT ----------------
        @blk.scalar
        def _(s):
            def ex(n):
                b, i = divmod(n, NBLK)
                q = n % 4
                if n >= 4:
                    s.wait_ge(pe_pt, n - 3)          # p_sb 4-deep WAR
                if i == 0 and b >= 2:
                    s.wait_ge(dve_rs, 16 * (b - 1))  # SS WAR vs recip of b-2
                ac = s.activation(p_sb[q][:], pS[:, q, :], Exp,
                                  bias=NM[b % 2][:, i:i + 1],
                                  accum_out=SS[b % 2][:, i:i + 1])
                ac._wait_ge(dve_nm, 4 * b + n % 16 // 4 + 1)
                ac.then_inc(act_p, 1)

            def outcp(n):
                b, i = divmod(n, NBLK)
                k = n % 2
                s.wait_ge(dve_rs, n + 1)
                if i == 0 and b >= 2:
                    s.wait_ge(s_out, 16 * (b - 1))
                oc = s.mul(o_all[b % 2][:, i, :], pO[:, k, :], RS[b % 2][:, i:i + 1])
                oc._wait_ge(pe_o, n + 1)
                oc.then_inc(act_o, 1)

            def tail(b):
                # T1: E = exp(-NM - 100), accum esum
                s.wait_ge(dve_nm, 4 * (b + 1))
                if b >= 1:
                    s.wait_ge(pt_, b)        # E_all/esum WAR vs tail C of b-1
                s.activation(E_all[:], NM[b % 2][:], Exp, bias=c100[:], scale=-1.0,
                             accum_out=esum[:]).then_inc(at, 1)
                # T2: qc = pQC * rtot
                s.wait_ge(dt, 2 * b + 2)
                s.wait_ge(pt_, b + 1)
                if b >= 2:
                    s.wait_ge(s_qc, 16 * (b - 1))
                s.mul(qc_sb[b % 2][:], pM[0:1, 0:256], rtot[:]).then_inc(at, 1)

            for b in range(NB):
                for slot in range(NBLK + 8):
                    i = slot - 7
                    if 0 <= i <= NBLK - 1:
                        outcp(16 * b + i)
                    i = slot - 4
                    if 0 <= i <= NBLK - 1:
                        ex(16 * b + i)
                tail(b)

        # ---------------- SYNC: output DMAs ----------------
        @blk.sync
        def _(sy):
            for b in range(NB):
                sy.wait_ge(act_o, 16 * (b + 1))
                if b >= 1:
                    sy.wait_ge(s_out, 16 * b)
                sy.dma_start(o_d[b].rearrange("(i p) d -> p i d", p=128),
                             o_all[b % 2][:]).then_inc(s_out, 16)
                sy.wait_ge(at, 2 * b + 2)
                if b >= 1:
                    sy.wait_ge(s_qc, 16 * b)
                sy.dma_start(qc_d[b:b + 1, :], qc_sb[b % 2][:]).then_inc(s_qc, 16)

    return nc, es


_CACHE = {}


def _get_program():
    if "nc" not in _CACHE:
        nc, es = build_program()
        _CACHE["nc"] = nc
        _CACHE["es"] = es
    return _CACHE["nc"]


def kernel(context_repr, question_repr, context_len, question_len):
    context_repr = np.ascontiguousarray(np.asarray(context_repr, np.float32))
    question_repr = np.ascontiguousarray(np.asarray(question_repr, np.float32))
    context_len = np.asarray(context_len, np.int32)
    question_len = np.asarray(question_len, np.int32)

    cm = (np.arange(TC)[None, :] < context_len[:, None]).astype(np.float32)  # [B,Tc]
    qm = (np.arange(TQ)[None, :] < question_len[:, None]).astype(np.float32)  # [B,Tq]
    mcf = np.stack([SQ * cm, np.ones_like(cm)], axis=1)                      # [B,2,Tc]
    mqf = np.stack([SQ * qm, np.full_like(qm, NEG)], axis=1)                 # [B,2,Tq]
    ident = np.eye(128, dtype=np.float32)
    onesw = np.ones((128, 256), np.float32)
    c100 = np.full((128, 1), -100.0, np.float32)

    nc = _get_program()
    in_maps = []
    for core in range(NCORES):
        sl = slice(core * NB, (core + 1) * NB)
        in_maps.append({
            "c": context_repr[sl],
            "q": question_repr[sl],
            "mcf": np.ascontiguousarray(mcf[sl]),
            "mqf": np.ascontiguousarray(mqf[sl]),
            "ident": ident,
            "onesw": onesw,
            "c100": c100,
        })

    res = run_bass_kernel_spmd(nc, in_maps, list(range(NCORES)))
    out1 = np.concatenate([np.asarray(r["o"]).reshape(NB, TC, D) for r in res.results], axis=0)
    q2c = np.concatenate([np.asarray(r["qc"]).reshape(NB, TQ) for r in res.results], axis=0)
    out2 = np.ascontiguousarray(np.broadcast_to(q2c[:, None, :], (B, TC, D)))
    return out1, out2


# revision 16
# speedup vs baseline: 1.2462x; 1.0036x over previous
"""BiAttention TRN2 kernel: data-parallel over batch across 8 NeuronCores.

Self-contained: hardcodes B=32, Tc=2048, Tq=256, D=256, 8 cores, 4 batches/core.
Raw-bass software-pipelined kernel; f32r matmuls; exact power-of-two mask trick.
"""
import numpy as np

import concourse.bass as bass
from concourse import mybir
from concourse.bass_utils import run_bass_kernel_spmd

F32 = mybir.dt.float32
F32R = mybir.dt.float32r
BF16 = mybir.dt.bfloat16
Exp = mybir.ActivationFunctionType.Exp
AX = mybir.AxisListType
OP = mybir.AluOpType

B, TC, TQ, D = 32, 2048, 256, 256
NCORES = 8
NB = B // NCORES          # batches per core = 4
NBLK = TC // 128          # c-blocks per batch = 16
NEG = -(2.0 ** 96)
SQ = 2.0 ** 48


def build_program():
    nc = bass.Bass()
    c_d = nc.declare_dram_parameter("c", [NB, TC, D], F32, isOutput=False)
    q_d = nc.declare_dram_parameter("q", [NB, TQ, D], F32, isOutput=False)
    mc_d = nc.declare_dram_parameter("mcf", [NB, 2, TC], F32, isOutput=False)
    mq_d = nc.declare_dram_parameter("mqf", [NB, 2, TQ], F32, isOutput=False)
    id_d = nc.declare_dram_parameter("ident", [128, 128], F32, isOutput=False)
    onew_d = nc.declare_dram_parameter("onesw", [128, 256], F32, isOutput=False)
    c100_d = nc.declare_dram_parameter("c100", [128, 1], F32, isOutput=False)

    o_d = nc.declare_dram_parameter("o", [NB, TC, D], F32, isOutput=True)
    qc_d = nc.declare_dram_parameter("qc", [NB, TQ], F32, isOutput=True)

    from contextlib import ExitStack
    es = ExitStack()
    _ctr = [0]

    def sb(shape, dt, name=None):
        _ctr[0] += 1
        return es.enter_context(nc.sbuf_tensor(name or f"sb{_ctr[0]}", shape, dt))

    def ps(shape, dt, name=None):
        _ctr[0] += 1
        return es.enter_context(nc.psum_tensor(name or f"ps{_ctr[0]}", shape, dt))

    def sem(name):
        return es.enter_context(nc.semaphore(name))

    # ---- SBUF ----
    cb = [sb([128, NBLK, D], F32R) for _ in range(2)]      # C natural (f32r), per-batch parity
    qn = [sb([128, 2, D], F32R) for _ in range(2)]          # Q natural [q%128, qchunk, d]
    qtr = [sb([128, 2, TQ], F32R) for _ in range(2)]        # Q^T [d%128, dchunk, q]
    mcs = [sb([2, TC], F32R) for _ in range(2)]             # mask lhsT features
    mqs = [sb([2, TQ], F32R) for _ in range(2)]             # mask rhs features
    ident = sb([128, 128], F32R)
    ones_w = sb([128, 256], F32R)                           # all-ones (total-sum rhs)
    c100 = sb([128, 1], F32)                                # bias constant -100
    ctr = [sb([128, 2, 128], F32R) for _ in range(2)]       # C^T chunks, block parity
    ptr = [sb([128, 2, 128], BF16) for _ in range(2)]       # P^T chunks (bf16), block parity
    p_sb = [sb([128, TQ], BF16) for _ in range(4)]          # exp(S-m) (bf16), 4-deep
    qn_b = [sb([128, 2, D], BF16) for _ in range(2)]        # Q natural bf16 (mm2 rhs)
    ident_b = sb([128, 128], BF16)
    o_all = [sb([128, NBLK, D], F32) for _ in range(2)]     # output batch buffer
    NM = [sb([128, NBLK], F32) for _ in range(2)]           # -rowmax per block column
    SS = [sb([128, NBLK], F32) for _ in range(2)]           # rowsum per block column
    RS = [sb([128, NBLK], F32) for _ in range(2)]           # 1/rowsum
    E_all = sb([128, NBLK], F32R)                           # exp(m - 100) for q2c
    esum = sb([128, 1], F32)
    esum_r = sb([128, 1], F32R)
    t_sb = sb([1, 1], F32)
    rtot = sb([1, 1], F32)
    qc_sb = [sb([1, TQ], F32) for _ in range(2)]

    # ---- PSUM (bank-granular allocator: 8 banks total) ----
    pJ = [ps([128, 256], F32R) for _ in range(2)]   # C^T per block parity (1 bank each)
    pPT = [ps([128, 128], F32R) for _ in range(2)]  # P^T (bf16 via bitcast), 1 bank each
    pS = ps([128, 4, 256], F32)                     # sim quad (2 banks); QT prep borrows bank0 via f32r bitcast
    pO = ps([128, 2, 256], F32)                     # mm2 out, 2 slots in 1 bank
    # pM regions: pQC=[0:1,0:256], pTot=[0:1,256:512]
    pM = ps([128, 512], F32)

    sems = {}
    for name in ("s_cin", "s_out", "s_qc", "pe_ct", "pe_qt", "pe_s", "pe_pt",
                 "pe_o", "pt_", "dve_ctr", "dve_qtr", "dve_nm", "dve_rs",
                 "dve_ptr", "dt", "act_p", "act_o", "at", "s_misc"):
        sems[name] = sem(name)
    s_cin = sems["s_cin"]; s_out = sems["s_out"]; s_qc = sems["s_qc"]
    pe_ct = sems["pe_ct"]; pe_qt = sems["pe_qt"]; pe_s = sems["pe_s"]
    pe_pt = sems["pe_pt"]; pe_o = sems["pe_o"]; pt_ = sems["pt_"]
    dve_ctr = sems["dve_ctr"]; dve_qtr = sems["dve_qtr"]; dve_nm = sems["dve_nm"]
    dve_rs = sems["dve_rs"]; dve_ptr = sems["dve_ptr"]; dt = sems["dt"]
    act_p = sems["act_p"]; act_o = sems["act_o"]; at = sems["at"]
    s_misc = sems["s_misc"]

    blk = es.enter_context(nc.Block())
    with blk:
        # ---------------- GPSIMD: input cast-DMAs ----------------
        @blk.gpsimd
        def _(g):
            for b in range(NB):
                if b >= 2:
                    g.wait_ge(pt_, b - 1)
                if b >= 1:
                    # all previously issued input DMAs must have completed so
                    # cumulative thresholds are meaningful (unordered DMA completion)
                    g.wait_ge(s_cin, 64 * b + 48)
                g.dma_start(cb[b % 2][:], c_d[b].rearrange("(i p) d -> p i d", p=128)).then_inc(s_cin, 16)
                g.dma_start(qn[b % 2][:], q_d[b].rearrange("(a p) d -> p a d", p=128)).then_inc(s_cin, 16)
                g.dma_start(mcs[b % 2][:], mc_d[b]).then_inc(s_cin, 16)
                g.dma_start(mqs[b % 2][:], mq_d[b]).then_inc(s_cin, 16)
                if b == 0:
                    g.dma_start(ident[:], id_d[:]).then_inc(s_cin, 16)
                    g.dma_start(ones_w[:], onew_d[:]).then_inc(s_cin, 16)
                    g.dma_start(c100[:], c100_d[:]).then_inc(s_cin, 16)

        def cin_thresh(b):
            return 64 * (b + 1) + 48

        # ---------------- PE ----------------
        @blk.tensor
        def _(t):
            def ct_tr(n):
                b, i = divmod(n, NBLK)
                k = n % 2
                if i == 0:
                    t.wait_ge(s_cin, cin_thresh(b))
                tr0 = t.transpose(pJ[k][:, 0:128], cb[b % 2][:, i, 0:128], ident[:])
                if n >= 2:
                    tr0._wait_ge(dve_ctr, n - 1)
                t.transpose(pJ[k][:, 128:256], cb[b % 2][:, i, 128:256], ident[:]).then_inc(pe_ct, 1)

            def sim(n):
                b, i = divmod(n, NBLK)
                k = n % 2
                q = n % 4
                t.wait_ge(dve_ctr, n + 1)
                if i in (0, 1):
                    t.wait_ge(dve_qtr, b + 1)     # bank0 quarters held QT
                ap = n - 2 - (n % 2)              # exp of evicted/conflicting quarter done
                if ap >= 1:
                    t.wait_ge(act_p, ap)          # also implies dve_nm >= n//4 transitively
                elif n >= 4:
                    t.wait_ge(dve_nm, n // 4)
                mm0 = t.matmul(pS[:, q, :], mcs[b % 2][:, i * 128:(i + 1) * 128],
                               mqs[b % 2][:], start=True, stop=False)
                t.matmul(pS[:, q, :], ctr[k][:, 0], qtr[b % 2][:, 0], start=False, stop=False)
                t.matmul(pS[:, q, :], ctr[k][:, 1], qtr[b % 2][:, 1], start=False, stop=True).then_inc(pe_s, 1)

            def pt_tr(n):
                k = n % 2
                if n == 0:
                    t.wait_ge(s_misc, 1)    # ident_b ready
                if n >= 2:
                    t.wait_ge(dve_ptr, n - 1)
                ptb = pPT[k][:].bitcast(BF16)
                tr0 = t.transpose(ptb[:, 0:128], p_sb[n % 4][:, 0:128], ident_b[:])
                tr0._wait_ge(act_p, n + 1)
                t.transpose(ptb[:, 128:256], p_sb[n % 4][:, 128:256], ident_b[:]).then_inc(pe_pt, 1)

            def mm2(n):
                b, i = divmod(n, NBLK)
                k = n % 2
                if n >= 1:
                    t.wait_ge(act_o, n)       # pO single bank: outcp(n-1) done
                mm0 = t.matmul(pO[:, k, :], ptr[k][:, 0], qn_b[b % 2][:, 0], start=True, stop=False)
                mm0._wait_ge(dve_ptr, n + 1)
                t.matmul(pO[:, k, :], ptr[k][:, 1], qn_b[b % 2][:, 1], start=False, stop=True).then_inc(pe_o, 1)

            def qt_prep(b):
                t.wait_ge(s_cin, cin_thresh(b))
                if b >= 1:
                    t.wait_ge(dve_qtr, b)       # prev QT copy done
                    t.wait_ge(act_p, 16 * b)    # pS bank0 prior exps done
                    t.wait_ge(dve_nm, 4 * b)    # prior quad reads done
                psr = pS[:].bitcast(F32R)
                last = None
                for qa in range(2):
                    for kk in range(2):
                        last = t.transpose(
                            psr[:, kk, qa * 128:(qa + 1) * 128],
                            qn[b % 2][:, qa, kk * 128:(kk + 1) * 128],
                            ident[:],
                        )
                last.then_inc(pe_qt, 1)

            def tail(b):
                # C: q2c matmuls + total sum (constant-shift exp, no global max)
                t.wait_ge(dt, 2 * b + 1)      # esum_r ready
                t.wait_ge(at, 2 * b + 1)      # E_all ready
                if b >= 1:
                    t.wait_ge(at, 2 * b)      # T2(b-1) done reading pM
                for i in range(NBLK):
                    t.matmul(pM[0:1, 0:256], E_all[:, i:i + 1], cb[b % 2][:, i, :],
                             start=(i == 0), stop=(i == NBLK - 1))
                t.matmul(pM[0:1, 256:512], esum_r[:], ones_w[:], start=True,
                         stop=True).then_inc(pt_, 1)

            for b in range(NB):
                qt_prep(b)
                for slot in range(NBLK + 8):
                    i = slot - 5
                    if 0 <= i <= NBLK - 1:
                        pt_tr(16 * b + i)
                    i = slot - 6
                    if 0 <= i <= NBLK - 1:
                        mm2(16 * b + i)
                    i = slot
                    if 0 <= i <= NBLK - 1:
                        ct_tr(16 * b + i)
                    i = slot - 1
                    if 0 <= i <= NBLK - 1:
                        sim(16 * b + i)
                tail(b)

        # ---------------- DVE ----------------
        @blk.vector
        def _(v):
            def qtr_copy(b):
                if b == 0:
                    v.wait_ge(s_cin, cin_thresh(0))
                    v.tensor_copy(ident_b[:], ident[:]).then_inc(s_misc, 1)
                v.wait_ge(pe_qt, b + 1)
                if b >= 2:
                    v.wait_ge(pe_o, 16 * (b - 1))   # qn_b WAR (implies pe_s too)
                v.tensor_copy(qn_b[b % 2][:], qn[b % 2][:])
                v.tensor_copy(qtr[b % 2][:], pS[:].bitcast(F32R)[:, 0:2, :]).then_inc(dve_qtr, 1)

            def ctr_copy(n):
                k = n % 2
                if n >= 2:
                    v.wait_ge(pe_s, n - 1)
                cp = v.tensor_copy(ctr[k][:], pJ[k][:, 0:256])
                cp._wait_ge(pe_ct, n + 1)
                cp.then_inc(dve_ctr, 1)

            def nm_quad(b, qq):
                # one reduce for blocks 16b+4qq .. +3
                i4 = 4 * qq
                if qq == 0 and b >= 2:
                    v.wait_ge(at, 2 * (b - 2) + 1)   # tail(b-2) E-exp read NM buffer
                rd = v.tensor_reduce(NM[b % 2][:, i4:i4 + 4], pS[:], AX.X, OP.max,
                                     negate=True)
                rd._wait_ge(pe_s, 16 * b + 4 * qq + 4)
                rd.then_inc(dve_nm, 1)

            def ptr_copy(n):
                k = n % 2
                if n >= 2:
                    v.wait_ge(pe_o, n - 1)
                cp = v.tensor_copy(ptr[k][:], pPT[k][:].bitcast(BF16)[:, 0:256])
                cp._wait_ge(pe_pt, n + 1)
                cp.then_inc(dve_ptr, 1)

            def recip(n):
                b, i = divmod(n, NBLK)
                if i == 0 and b >= 2:
                    v.wait_ge(act_o, 16 * (b - 1))   # RS WAR vs out-copy of b-2
                rc = v.reciprocal(RS[b % 2][:, i:i + 1], SS[b % 2][:, i:i + 1])
                rc._wait_ge(act_p, n + 1)
                rc.then_inc(dve_rs, 1)

            def tail(b):
                # X1: esum -> f32r
                v.wait_ge(at, 2 * b + 1)
                v.tensor_copy(esum_r[:], esum[:]).then_inc(dt, 1)
                # X2: total -> reciprocal
                v.wait_ge(pt_, b + 1)
                if b >= 1:
                    v.wait_ge(at, 2 * b)   # T2(b-1) done with rtot
                v.tensor_copy(t_sb[:], pM[0:1, 256:257])
                v.drain()
                v.reciprocal(rtot[:], t_sb[:]).then_inc(dt, 1)

            for b in range(NB):
                qtr_copy(b)
                for slot in range(NBLK + 8):
                    i = slot - 6
                    if 0 <= i <= NBLK - 1:
                        ptr_copy(16 * b + i)
                    i = slot - 5
                    if 0 <= i <= NBLK - 1:
                        recip(16 * b + i)
                    i = slot
                    if 0 <= i <= NBLK - 1:
                        ctr_copy(16 * b + i)
                    if slot >= 4 and (slot - 4) % 4 == 0 and (slot - 4) // 4 <= 3:
                        nm_quad(b, (slot - 4) // 4)
                tail(b)

        # ---------------- ACT ----------------
        @blk.scalar
        def _(s):
            def ex(n):
                b, i = divmod(n, NBLK)
                q = n % 4
                if n >= 4:
                    s.wait_ge(pe_pt, n - 3)          # p_sb 4-deep WAR
                if i == 0 and b >= 2:
                    s.wait_ge(dve_rs, 16 * (b - 1))  # SS WAR vs recip of b-2
                ac = s.activation(p_sb[q][:], pS[:, q, :], Exp,
                                  bias=NM[b % 2][:, i:i + 1],
                                  accum_out=SS[b % 2][:, i:i + 1])
                ac._wait_ge(dve_nm, 4 * b + n % 16 // 4 + 1)
                ac.then_inc(act_p, 1)

            def outcp(n):
                b, i = divmod(n, NBLK)
                k = n % 2
                s.wait_ge(dve_rs, n + 1)
                if i == 0 and b >= 2:
                    s.wait_ge(s_out, 16 * (b - 1))
                oc = s.mul(o_all[b % 2][:, i, :], pO[:, k, :], RS[b % 2][:, i:i + 1])
                oc._wait_ge(pe_o, n + 1)
                oc.then_inc(act_o, 1)

            def tail(b):
                # T1: E = exp(-NM - 100), accum esum
                s.wait_ge(dve_nm, 4 * (b + 1))
                if b >= 1:
                    s.wait_ge(pt_, b)        # E_all/esum WAR vs tail C of b-1
                s.activation(E_all[:], NM[b % 2][:], Exp, bias=c100[:], scale=-1.0,
                             accum_out=esum[:]).then_inc(at, 1)
                # T2: qc = pQC * rtot
                s.wait_ge(dt, 2 * b + 2)
                s.wait_ge(pt_, b + 1)
                if b >= 2:
                    s.wait_ge(s_qc, 16 * (b - 1))
                s.mul(qc_sb[b % 2][:], pM[0:1, 0:256], rtot[:]).then_inc(at, 1)

            for b in range(NB):
                for slot in range(NBLK + 8):
                    i = slot - 7
                    if 0 <= i <= NBLK - 1:
                        outcp(16 * b + i)
                    i = slot - 4
                    if 0 <= i <= NBLK - 1:
                        ex(16 * b + i)
                tail(b)

        # ---------------- SYNC: output DMAs ----------------
        @blk.sync
        def _(sy):
            for b in range(NB):
                sy.wait_ge(act_o, 16 * (b + 1))
                if b >= 1:
                    sy.wait_ge(s_out, 16 * b)
                sy.dma_start(o_d[b].rearrange("(i p) d -> p i d", p=128),
                             o_all[b % 2][:]).then_inc(s_out, 16)
                sy.wait_ge(at, 2 * b + 2)
                if b >= 1:
                    sy.wait_ge(s_qc, 16 * b)
                sy.dma_start(qc_d[b:b + 1, :], qc_sb[b % 2][:]).then_inc(s_qc, 16)

    return nc, es


_CACHE = {}


def _get_program():
    if "nc" not in _CACHE:
        nc, es = build_program()
        _CACHE["nc"] = nc
        _CACHE["es"] = es
    return _CACHE["nc"]


def kernel(context_repr, question_repr, context_len, question_len):
    context_repr = np.ascontiguousarray(np.asarray(context_repr, np.float32))
    question_repr = np.ascontiguousarray(np.asarray(question_repr, np.float32))
    context_len = np.asarray(context_len, np.int32)
    question_len = np.asarray(question_len, np.int32)

    cm = (np.arange(TC)[None, :] < context_len[:, None]).astype(np.float32)  # [B,Tc]
    qm = (np.arange(TQ)[None, :] < question_len[:, None]).astype(np.float32)  # [B,Tq]
    mcf = np.stack([SQ * cm, np.ones_like(cm)], axis=1)                      # [B,2,Tc]
    mqf = np.stack([SQ * qm, np.full_like(qm, NEG)], axis=1)                 # [B,2,Tq]
    ident = np.eye(128, dtype=np.float32)
    onesw = np.ones((128, 256), np.float32)
    c100 = np.full((128, 1), -100.0, np.float32)

    nc = _get_program()
    in_maps = []
    for core in range(NCORES):
        sl = slice(core * NB, (core + 1) * NB)
        in_maps.append({
            "c": context_repr[sl],
            "q": question_repr[sl],
            "mcf": np.ascontiguousarray(mcf[sl]),
            "mqf": np.ascontiguousarray(mqf[sl]),
            "ident": ident,
            "onesw": onesw,
            "c100": c100,
        })

    res = run_bass_kernel_spmd(nc, in_maps, list(range(NCORES)))
    out1 = np.concatenate([np.asarray(r["o"]).reshape(NB, TC, D) for r in res.results], axis=0)
    q2c = np.concatenate([np.asarray(r["qc"]).reshape(NB, TQ) for r in res.results], axis=0)
    out2 = np.ascontiguousarray(np.broadcast_to(q2c[:, None, :], (B, TC, D)))
    return out1, out2


# revision 19
# speedup vs baseline: 1.2807x; 1.0277x over previous
"""BiAttention TRN2 kernel: data-parallel over batch across 8 NeuronCores.

Self-contained: hardcodes B=32, Tc=2048, Tq=256, D=256, 8 cores, 4 batches/core.
Raw-bass software-pipelined kernel; f32r matmuls; exact power-of-two mask trick.
"""
import numpy as np

import concourse.bass as bass
from concourse import mybir
from concourse.bass_utils import run_bass_kernel_spmd

F32 = mybir.dt.float32
F32R = mybir.dt.float32r
BF16 = mybir.dt.bfloat16
Exp = mybir.ActivationFunctionType.Exp
AX = mybir.AxisListType
OP = mybir.AluOpType

B, TC, TQ, D = 32, 2048, 256, 256
NCORES = 8
NB = B // NCORES          # batches per core = 4
NBLK = TC // 128          # c-blocks per batch = 16
NEG = -(2.0 ** 96)
SQ = 2.0 ** 48


def build_program():
    nc = bass.Bass()
    c_d = nc.declare_dram_parameter("c", [NB, TC, D], F32, isOutput=False)
    q_d = nc.declare_dram_parameter("q", [NB, TQ, D], F32, isOutput=False)
    mc_d = nc.declare_dram_parameter("mcf", [NB, 2, TC], F32, isOutput=False)
    mq_d = nc.declare_dram_parameter("mqf", [NB, 2, TQ], F32, isOutput=False)
    id_d = nc.declare_dram_parameter("ident", [128, 128], F32, isOutput=False)
    onew_d = nc.declare_dram_parameter("onesw", [128, 256], F32, isOutput=False)
    c100_d = nc.declare_dram_parameter("c100", [128, 1], F32, isOutput=False)

    o_d = nc.declare_dram_parameter("o", [NB, TC, D], F32, isOutput=True)
    qc_d = nc.declare_dram_parameter("qc", [NB, TQ], F32, isOutput=True)

    from contextlib import ExitStack
    es = ExitStack()
    _ctr = [0]

    def sb(shape, dt, name=None):
        _ctr[0] += 1
        return es.enter_context(nc.sbuf_tensor(name or f"sb{_ctr[0]}", shape, dt))

    def ps(shape, dt, name=None):
        _ctr[0] += 1
        return es.enter_context(nc.psum_tensor(name or f"ps{_ctr[0]}", shape, dt))

    def sem(name):
        return es.enter_context(nc.semaphore(name))

    # ---- SBUF ----
    cb = [sb([128, NBLK, D], F32R) for _ in range(2)]      # C natural (f32r), per-batch parity
    qn = [sb([128, 2, D], F32R) for _ in range(2)]          # Q natural [q%128, qchunk, d]
    qtr = [sb([128, 2, TQ], F32R) for _ in range(2)]        # Q^T [d%128, dchunk, q]
    mcs = [sb([2, TC], F32R) for _ in range(2)]             # mask lhsT features
    mqs = [sb([2, TQ], F32R) for _ in range(2)]             # mask rhs features
    ident = sb([128, 128], F32R)
    ones_w = sb([128, 256], F32R)                           # all-ones (total-sum rhs)
    c100 = sb([128, 1], F32)                                # bias constant -100
    ctr = [sb([128, 2, 2, 128], F32R) for _ in range(2)]    # C^T (par, chunk, c), pair-parity
    ptr = [sb([128, 2, 128], BF16) for _ in range(2)]       # P^T chunks (bf16), block parity
    p_sb = [sb([128, TQ], BF16) for _ in range(4)]          # exp(S-m) (bf16), 4-deep
    qn_b = [sb([128, 2, D], BF16) for _ in range(2)]        # Q natural bf16 (mm2 rhs)
    ident_b = sb([128, 128], BF16)
    o_all = [sb([128, NBLK, D], F32) for _ in range(2)]     # output batch buffer
    NM = [sb([128, NBLK], F32) for _ in range(2)]           # -rowmax per block column
    SS = [sb([128, NBLK], F32) for _ in range(2)]           # rowsum per block column
    RS = [sb([128, NBLK], F32) for _ in range(2)]           # 1/rowsum
    E_all = sb([128, NBLK], F32R)                           # exp(m - 100) for q2c
    esum = sb([128, 1], F32)
    esum_r = sb([128, 1], F32R)
    t_sb = sb([1, 1], F32)
    rtot = sb([1, 1], F32)
    qc_sb = [sb([1, TQ], F32) for _ in range(2)]

    # ---- PSUM (bank-granular allocator: 8 banks total) ----
    pJ = ps([128, 2, 256], F32R)                    # C^T both parities in 1 bank
    pPT = [ps([128, 128], F32R) for _ in range(2)]  # P^T (bf16 via bitcast), 1 bank each
    pS = ps([128, 4, 256], F32)                     # sim quad (2 banks); QT prep borrows bank0 via f32r bitcast
    pO = [ps([128, 256], F32) for _ in range(2)]    # mm2 out, 1 bank each
    # pM regions: pQC=[0:1,0:256], pTot=[0:1,256:512]
    pM = ps([128, 512], F32)

    sems = {}
    for name in ("s_cin", "s_out", "s_qc", "pe_ct", "pe_qt", "pe_s", "pe_pt",
                 "pe_o", "pt_", "dve_ctr", "dve_qtr", "dve_nm", "dve_rs",
                 "dve_ptr", "dt", "act_p", "act_o", "at", "s_misc"):
        sems[name] = sem(name)
    s_cin = sems["s_cin"]; s_out = sems["s_out"]; s_qc = sems["s_qc"]
    pe_ct = sems["pe_ct"]; pe_qt = sems["pe_qt"]; pe_s = sems["pe_s"]
    pe_pt = sems["pe_pt"]; pe_o = sems["pe_o"]; pt_ = sems["pt_"]
    dve_ctr = sems["dve_ctr"]; dve_qtr = sems["dve_qtr"]; dve_nm = sems["dve_nm"]
    dve_rs = sems["dve_rs"]; dve_ptr = sems["dve_ptr"]; dt = sems["dt"]
    act_p = sems["act_p"]; act_o = sems["act_o"]; at = sems["at"]
    s_misc = sems["s_misc"]

    blk = es.enter_context(nc.Block())
    with blk:
        # ---------------- GPSIMD: input cast-DMAs ----------------
        @blk.gpsimd
        def _(g):
            for b in range(NB):
                if b >= 2:
                    g.wait_ge(pt_, b - 1)
                if b >= 1:
                    # all previously issued input DMAs must have completed so
                    # cumulative thresholds are meaningful (unordered DMA completion)
                    g.wait_ge(s_cin, 64 * b + 48)
                g.dma_start(cb[b % 2][:], c_d[b].rearrange("(i p) d -> p i d", p=128)).then_inc(s_cin, 16)
                g.dma_start(qn[b % 2][:], q_d[b].rearrange("(a p) d -> p a d", p=128)).then_inc(s_cin, 16)
                g.dma_start(mcs[b % 2][:], mc_d[b]).then_inc(s_cin, 16)
                g.dma_start(mqs[b % 2][:], mq_d[b]).then_inc(s_cin, 16)
                if b == 0:
                    g.dma_start(ident[:], id_d[:]).then_inc(s_cin, 16)
                    g.dma_start(ones_w[:], onew_d[:]).then_inc(s_cin, 16)
                    g.dma_start(c100[:], c100_d[:]).then_inc(s_cin, 16)

        def cin_thresh(b):
            return 64 * (b + 1) + 48

        # ---------------- PE ----------------
        @blk.tensor
        def _(t):
            def ct_tr(n):
                b, i = divmod(n, NBLK)
                k = n % 2
                if i == 0:
                    t.wait_ge(s_cin, cin_thresh(b))
                tr0 = t.transpose(pJ[:, k, 0:128], cb[b % 2][:, i, 0:128], ident[:])
                if n >= 2:
                    tr0._wait_ge(dve_ctr, n // 2)   # pair copy of (n-2) done (whole bank)
                t.transpose(pJ[:, k, 128:256], cb[b % 2][:, i, 128:256], ident[:]).then_inc(pe_ct, 1)

            def sim(n):
                b, i = divmod(n, NBLK)
                k = n % 2
                q = n % 4
                t.wait_ge(dve_ctr, n // 2 + 1)
                if i in (0, 1):
                    t.wait_ge(dve_qtr, b + 1)     # bank0 quarters held QT
                ap = n - 2 - (n % 2)              # exp of evicted/conflicting quarter done
                if ap >= 1:
                    t.wait_ge(act_p, ap)          # also implies dve_nm >= n//4 transitively
                elif n >= 4:
                    t.wait_ge(dve_nm, n // 4)
                mm0 = t.matmul(pS[:, q, :], mcs[b % 2][:, i * 128:(i + 1) * 128],
                               mqs[b % 2][:], start=True, stop=False)
                pp = (n // 2) % 2
                t.matmul(pS[:, q, :], ctr[pp][:, k, 0], qtr[b % 2][:, 0], start=False, stop=False)
                t.matmul(pS[:, q, :], ctr[pp][:, k, 1], qtr[b % 2][:, 1], start=False, stop=True).then_inc(pe_s, 1)

            def pt_tr(n):
                k = n % 2
                if n == 0:
                    t.wait_ge(s_misc, 1)    # ident_b ready
                if n >= 2:
                    t.wait_ge(dve_ptr, n - 1)
                ptb = pPT[k][:].bitcast(BF16)
                tr0 = t.transpose(ptb[:, 0:128], p_sb[n % 4][:, 0:128], ident_b[:])
                tr0._wait_ge(act_p, n + 1)
                t.transpose(ptb[:, 128:256], p_sb[n % 4][:, 128:256], ident_b[:]).then_inc(pe_pt, 1)

            def mm2(n):
                b, i = divmod(n, NBLK)
                k = n % 2
                if n >= 2:
                    t.wait_ge(act_o, n - 1)   # outcp(n-2) done (own bank)
                mm0 = t.matmul(pO[k][:], ptr[k][:, 0], qn_b[b % 2][:, 0], start=True, stop=False)
                mm0._wait_ge(dve_ptr, n + 1)
                t.matmul(pO[k][:], ptr[k][:, 1], qn_b[b % 2][:, 1], start=False, stop=True).then_inc(pe_o, 1)

            def qt_prep(b):
                t.wait_ge(s_cin, cin_thresh(b))
                if b >= 1:
                    t.wait_ge(dve_qtr, b)       # prev QT copy done
                    t.wait_ge(act_p, 16 * b)    # pS bank0 prior exps done
                    t.wait_ge(dve_nm, 4 * b)    # prior quad reads done
                psr = pS[:].bitcast(F32R)
                last = None
                for qa in range(2):
                    for kk in range(2):
                        last = t.transpose(
                            psr[:, kk, qa * 128:(qa + 1) * 128],
                            qn[b % 2][:, qa, kk * 128:(kk + 1) * 128],
                            ident[:],
                        )
                last.then_inc(pe_qt, 1)

            def tail(b):
                # C: q2c matmuls + total sum (constant-shift exp, no global max)
                t.wait_ge(dt, 2 * b + 1)      # esum_r ready
                t.wait_ge(at, 2 * b + 1)      # E_all ready
                if b >= 1:
                    t.wait_ge(at, 2 * b)      # T2(b-1) done reading pM
                for i in range(NBLK):
                    t.matmul(pM[0:1, 0:256], E_all[:, i:i + 1], cb[b % 2][:, i, :],
                             start=(i == 0), stop=(i == NBLK - 1))
                t.matmul(pM[0:1, 256:512], esum_r[:], ones_w[:], start=True,
                         stop=True).then_inc(pt_, 1)

            for b in range(NB):
                qt_prep(b)
                for slot in range(NBLK + 10):
                    i = slot - 6
                    if 0 <= i <= NBLK - 1:
                        pt_tr(16 * b + i)
                    i = slot - 7
                    if 0 <= i <= NBLK - 1:
                        mm2(16 * b + i)
                    i = slot
                    if 0 <= i <= NBLK - 1:
                        ct_tr(16 * b + i)
                    i = slot - 2
                    if 0 <= i <= NBLK - 1:
                        sim(16 * b + i)
                tail(b)

        # ---------------- DVE ----------------
        @blk.vector
        def _(v):
            def qtr_copy(b):
                if b == 0:
                    v.wait_ge(s_cin, cin_thresh(0))
                    v.tensor_copy(ident_b[:], ident[:]).then_inc(s_misc, 1)
                v.wait_ge(pe_qt, b + 1)
                if b >= 2:
                    v.wait_ge(pe_o, 16 * (b - 1))   # qn_b WAR (implies pe_s too)
                v.tensor_copy(qn_b[b % 2][:], qn[b % 2][:])
                v.tensor_copy(qtr[b % 2][:], pS[:].bitcast(F32R)[:, 0:2, :]).then_inc(dve_qtr, 1)

            def ctr_pair(b, p):
                # copy C^T for blocks 16b+2p, +2p+1 in one op
                n1 = 16 * b + 2 * p + 1
                if n1 >= 5:
                    v.wait_ge(pe_s, n1 - 3)       # sims of pair evicted 2 pairs ago done
                cp = v.tensor_copy(ctr[p % 2][:], pJ[:])
                cp._wait_ge(pe_ct, n1 + 1)
                cp.then_inc(dve_ctr, 1)

            def nm_quad(b, qq):
                # one reduce for blocks 16b+4qq .. +3
                i4 = 4 * qq
                if qq == 0 and b >= 2:
                    v.wait_ge(at, 2 * (b - 2) + 1)   # tail(b-2) E-exp read NM buffer
                rd = v.tensor_reduce(NM[b % 2][:, i4:i4 + 4], pS[:], AX.X, OP.max,
                                     negate=True)
                rd._wait_ge(pe_s, 16 * b + 4 * qq + 4)
                rd.then_inc(dve_nm, 1)

            def ptr_copy(n):
                k = n % 2
                if n >= 2:
                    v.wait_ge(pe_o, n - 1)
                cp = v.tensor_copy(ptr[k][:], pPT[k][:].bitcast(BF16)[:, 0:256])
                cp._wait_ge(pe_pt, n + 1)
                cp.then_inc(dve_ptr, 1)

            def recip(n):
                b, i = divmod(n, NBLK)
                if i == 0 and b >= 2:
                    v.wait_ge(act_o, 16 * (b - 1))   # RS WAR vs out-copy of b-2
                rc = v.reciprocal(RS[b % 2][:, i:i + 1], SS[b % 2][:, i:i + 1])
                rc._wait_ge(act_p, n + 1)
                rc.then_inc(dve_rs, 1)

            def tail(b):
                # X1: esum -> f32r
                v.wait_ge(at, 2 * b + 1)
                v.tensor_copy(esum_r[:], esum[:]).then_inc(dt, 1)
                # X2: total -> reciprocal
                v.wait_ge(pt_, b + 1)
                if b >= 1:
                    v.wait_ge(at, 2 * b)   # T2(b-1) done with rtot
                v.tensor_copy(t_sb[:], pM[0:1, 256:257])
                v.drain()
                v.reciprocal(rtot[:], t_sb[:]).then_inc(dt, 1)

            for b in range(NB):
                qtr_copy(b)
                for slot in range(NBLK + 10):
                    i = slot - 7
                    if 0 <= i <= NBLK - 1:
                        ptr_copy(16 * b + i)
                    i = slot - 6
                    if 0 <= i <= NBLK - 1:
                        recip(16 * b + i)
                    if slot % 2 == 1 and (slot - 1) // 2 <= 7:
                        ctr_pair(b, (slot - 1) // 2)
                    if slot >= 5 and (slot - 5) % 4 == 0 and (slot - 5) // 4 <= 3:
                        nm_quad(b, (slot - 5) // 4)
                tail(b)

        # ---------------- ACT ----------------
        @blk.scalar
        def _(s):
            def ex(n):
                b, i = divmod(n, NBLK)
                q = n % 4
                if n >= 4:
                    s.wait_ge(pe_pt, n - 3)          # p_sb 4-deep WAR
                if i == 0 and b >= 2:
                    s.wait_ge(dve_rs, 16 * (b - 1))  # SS WAR vs recip of b-2
                ac = s.activation(p_sb[q][:], pS[:, q, :], Exp,
                                  bias=NM[b % 2][:, i:i + 1],
                                  accum_out=SS[b % 2][:, i:i + 1])
                ac._wait_ge(dve_nm, 4 * b + n % 16 // 4 + 1)
                ac.then_inc(act_p, 1)

            def outcp(n):
                b, i = divmod(n, NBLK)
                k = n % 2
                s.wait_ge(dve_rs, n + 1)
                if i == 0 and b >= 2:
                    s.wait_ge(s_out, 16 * (b - 1))
                oc = s.mul(o_all[b % 2][:, i, :], pO[k][:], RS[b % 2][:, i:i + 1])
                oc._wait_ge(pe_o, n + 1)
                oc.then_inc(act_o, 1)

            def tail(b):
                # T1: E = exp(-NM - 100), accum esum
                s.wait_ge(dve_nm, 4 * (b + 1))
                if b >= 1:
                    s.wait_ge(pt_, b)        # E_all/esum WAR vs tail C of b-1
                s.activation(E_all[:], NM[b % 2][:], Exp, bias=c100[:], scale=-1.0,
                             accum_out=esum[:]).then_inc(at, 1)
                # T2: qc = pQC * rtot
                s.wait_ge(dt, 2 * b + 2)
                s.wait_ge(pt_, b + 1)
                if b >= 2:
                    s.wait_ge(s_qc, 16 * (b - 1))
                s.mul(qc_sb[b % 2][:], pM[0:1, 0:256], rtot[:]).then_inc(at, 1)

            for b in range(NB):
                for slot in range(NBLK + 10):
                    i = slot - 8
                    if 0 <= i <= NBLK - 1:
                        outcp(16 * b + i)
                    i = slot - 4
                    if 0 <= i <= NBLK - 1:
                        ex(16 * b + i)
                tail(b)

        # ---------------- SYNC: output DMAs ----------------
        @blk.sync
        def _(sy):
            for b in range(NB):
                sy.wait_ge(act_o, 16 * (b + 1))
                if b >= 1:
                    sy.wait_ge(s_out, 16 * b)
                sy.dma_start(o_d[b].rearrange("(i p) d -> p i d", p=128),
                             o_all[b % 2][:]).then_inc(s_out, 16)
                sy.wait_ge(at, 2 * b + 2)
                if b >= 1:
                    sy.wait_ge(s_qc, 16 * b)
                sy.dma_start(qc_d[b:b + 1, :], qc_sb[b % 2][:]).then_inc(s_qc, 16)

    return nc, es


_CACHE = {}


def _get_program():
    if "nc" not in _CACHE:
        nc, es = build_program()
        _CACHE["nc"] = nc
        _CACHE["es"] = es
    return _CACHE["nc"]


def kernel(context_repr, question_repr, context_len, question_len):
    context_repr = np.ascontiguousarray(np.asarray(context_repr, np.float32))
    question_repr = np.ascontiguousarray(np.asarray(question_repr, np.float32))
    context_len = np.asarray(context_len, np.int32)
    question_len = np.asarray(question_len, np.int32)

    cm = (np.arange(TC)[None, :] < context_len[:, None]).astype(np.float32)  # [B,Tc]
    qm = (np.arange(TQ)[None, :] < question_len[:, None]).astype(np.float32)  # [B,Tq]
    mcf = np.stack([SQ * cm, np.ones_like(cm)], axis=1)                      # [B,2,Tc]
    mqf = np.stack([SQ * qm, np.full_like(qm, NEG)], axis=1)                 # [B,2,Tq]
    ident = np.eye(128, dtype=np.float32)
    onesw = np.ones((128, 256), np.float32)
    c100 = np.full((128, 1), -100.0, np.float32)

    nc = _get_program()
    in_maps = []
    for core in range(NCORES):
        sl = slice(core * NB, (core + 1) * NB)
        in_maps.append({
            "c": context_repr[sl],
            "q": question_repr[sl],
            "mcf": np.ascontiguousarray(mcf[sl]),
            "mqf": np.ascontiguousarray(mqf[sl]),
            "ident": ident,
            "onesw": onesw,
            "c100": c100,
        })

    res = run_bass_kernel_spmd(nc, in_maps, list(range(NCORES)))
    out1 = np.concatenate([np.asarray(r["o"]).reshape(NB, TC, D) for r in res.results], axis=0)
    q2c = np.concatenate([np.asarray(r["qc"]).reshape(NB, TQ) for r in res.results], axis=0)
    out2 = np.ascontiguousarray(np.broadcast_to(q2c[:, None, :], (B, TC, D)))
    return out1, out2


# revision 20
# speedup vs baseline: 1.2978x; 1.0133x over previous
"""BiAttention TRN2 kernel: data-parallel over batch across 8 NeuronCores.

Self-contained: hardcodes B=32, Tc=2048, Tq=256, D=256, 8 cores, 4 batches/core.
Raw-bass software-pipelined kernel; f32r matmuls; exact power-of-two mask trick.
"""
import numpy as np

import concourse.bass as bass
from concourse import mybir
from concourse.bass_utils import run_bass_kernel_spmd

F32 = mybir.dt.float32
F32R = mybir.dt.float32r
BF16 = mybir.dt.bfloat16
Exp = mybir.ActivationFunctionType.Exp
AX = mybir.AxisListType
OP = mybir.AluOpType

B, TC, TQ, D = 32, 2048, 256, 256
NCORES = 8
NB = B // NCORES          # batches per core = 4
NBLK = TC // 128          # c-blocks per batch = 16
NEG = -(2.0 ** 96)
SQ = 2.0 ** 48


def build_program():
    nc = bass.Bass()
    c_d = nc.declare_dram_parameter("c", [NB, TC, D], F32, isOutput=False)
    q_d = nc.declare_dram_parameter("q", [NB, TQ, D], F32, isOutput=False)
    mc_d = nc.declare_dram_parameter("mcf", [NB, 2, TC], F32, isOutput=False)
    mq_d = nc.declare_dram_parameter("mqf", [NB, 2, TQ], F32, isOutput=False)
    id_d = nc.declare_dram_parameter("ident", [128, 128], F32, isOutput=False)
    onew_d = nc.declare_dram_parameter("onesw", [128, 256], F32, isOutput=False)
    c100_d = nc.declare_dram_parameter("c100", [128, 1], F32, isOutput=False)

    o_d = nc.declare_dram_parameter("o", [NB, TC, D], F32, isOutput=True)
    qc_d = nc.declare_dram_parameter("qc", [NB, TQ], F32, isOutput=True)

    from contextlib import ExitStack
    es = ExitStack()
    _ctr = [0]

    def sb(shape, dt, name=None):
        _ctr[0] += 1
        return es.enter_context(nc.sbuf_tensor(name or f"sb{_ctr[0]}", shape, dt))

    def ps(shape, dt, name=None):
        _ctr[0] += 1
        return es.enter_context(nc.psum_tensor(name or f"ps{_ctr[0]}", shape, dt))

    def sem(name):
        return es.enter_context(nc.semaphore(name))

    # ---- SBUF ----
    cb = [sb([128, NBLK, D], F32R) for _ in range(2)]      # C natural (f32r), per-batch parity
    qn = [sb([128, 2, D], F32R) for _ in range(2)]          # Q natural [q%128, qchunk, d]
    qtr = [sb([128, 2, TQ], F32R) for _ in range(2)]        # Q^T [d%128, dchunk, q]
    mcs = [sb([2, TC], F32R) for _ in range(2)]             # mask lhsT features
    mqs = [sb([2, TQ], F32R) for _ in range(2)]             # mask rhs features
    ident = sb([128, 128], F32R)
    ones_w = sb([128, 256], F32R)                           # all-ones (total-sum rhs)
    c100 = sb([128, 1], F32)                                # bias constant -100
    ctr = [sb([128, 2, 2, 128], F32R) for _ in range(2)]    # C^T (par, chunk, c), pair-parity
    ptr = [sb([128, 2, 2, 128], BF16) for _ in range(2)]    # P^T (par, chunk, c), pair-parity
    p_sb = [sb([128, TQ], BF16) for _ in range(4)]          # exp(S-m) (bf16), 4-deep
    qn_b = [sb([128, 2, D], BF16) for _ in range(2)]        # Q natural bf16 (mm2 rhs)
    ident_b = sb([128, 128], BF16)
    o_all = [sb([128, NBLK, D], F32) for _ in range(2)]     # output batch buffer
    NM = [sb([128, NBLK], F32) for _ in range(2)]           # -rowmax per block column
    SS = [sb([128, NBLK], F32) for _ in range(2)]           # rowsum per block column
    RS = [sb([128, NBLK], F32) for _ in range(2)]           # 1/rowsum
    E_all = sb([128, NBLK], F32R)                           # exp(m - 100) for q2c
    esum = sb([128, 1], F32)
    esum_r = sb([128, 1], F32R)
    t_sb = sb([1, 1], F32)
    rtot = sb([1, 1], F32)
    qc_sb = [sb([1, TQ], F32) for _ in range(2)]

    # ---- PSUM (bank-granular allocator: 8 banks total) ----
    pJ = ps([128, 2, 256], F32R)                    # C^T both parities in 1 bank
    pPT = ps([128, 256], F32R)                      # P^T both parities (bf16 via bitcast), 1 bank
    pS = ps([128, 4, 256], F32)                     # sim quad (2 banks); QT prep borrows bank0 via f32r bitcast
    pO = [ps([128, 256], F32) for _ in range(2)]    # mm2 out, 1 bank each
    # pM regions: pQC=[0:1,0:256], pTot=[0:1,256:512]
    pM = ps([128, 512], F32)

    sems = {}
    for name in ("s_cin", "s_out", "s_qc", "pe_ct", "pe_qt", "pe_s", "pe_pt",
                 "pe_o", "pt_", "dve_ctr", "dve_qtr", "dve_nm", "dve_rs",
                 "dve_ptr", "dt", "act_p", "act_o", "at", "s_misc"):
        sems[name] = sem(name)
    s_cin = sems["s_cin"]; s_out = sems["s_out"]; s_qc = sems["s_qc"]
    pe_ct = sems["pe_ct"]; pe_qt = sems["pe_qt"]; pe_s = sems["pe_s"]
    pe_pt = sems["pe_pt"]; pe_o = sems["pe_o"]; pt_ = sems["pt_"]
    dve_ctr = sems["dve_ctr"]; dve_qtr = sems["dve_qtr"]; dve_nm = sems["dve_nm"]
    dve_rs = sems["dve_rs"]; dve_ptr = sems["dve_ptr"]; dt = sems["dt"]
    act_p = sems["act_p"]; act_o = sems["act_o"]; at = sems["at"]
    s_misc = sems["s_misc"]

    blk = es.enter_context(nc.Block())
    with blk:
        # ---------------- GPSIMD: input cast-DMAs ----------------
        @blk.gpsimd
        def _(g):
            for b in range(NB):
                if b >= 2:
                    g.wait_ge(pt_, b - 1)
                if b >= 1:
                    # all previously issued input DMAs must have completed so
                    # cumulative thresholds are meaningful (unordered DMA completion)
                    g.wait_ge(s_cin, 64 * b + 48)
                g.dma_start(cb[b % 2][:], c_d[b].rearrange("(i p) d -> p i d", p=128)).then_inc(s_cin, 16)
                g.dma_start(qn[b % 2][:], q_d[b].rearrange("(a p) d -> p a d", p=128)).then_inc(s_cin, 16)
                g.dma_start(mcs[b % 2][:], mc_d[b]).then_inc(s_cin, 16)
                g.dma_start(mqs[b % 2][:], mq_d[b]).then_inc(s_cin, 16)
                if b == 0:
                    g.dma_start(ident[:], id_d[:]).then_inc(s_cin, 16)
                    g.dma_start(ones_w[:], onew_d[:]).then_inc(s_cin, 16)
                    g.dma_start(c100[:], c100_d[:]).then_inc(s_cin, 16)

        def cin_thresh(b):
            return 64 * (b + 1) + 48

        # ---------------- PE ----------------
        @blk.tensor
        def _(t):
            def ct_tr(n):
                b, i = divmod(n, NBLK)
                k = n % 2
                if i == 0:
                    t.wait_ge(s_cin, cin_thresh(b))
                tr0 = t.transpose(pJ[:, k, 0:128], cb[b % 2][:, i, 0:128], ident[:])
                if n >= 2:
                    tr0._wait_ge(dve_ctr, n // 2)   # pair copy of (n-2) done (whole bank)
                t.transpose(pJ[:, k, 128:256], cb[b % 2][:, i, 128:256], ident[:]).then_inc(pe_ct, 1)

            def sim(n):
                b, i = divmod(n, NBLK)
                k = n % 2
                q = n % 4
                t.wait_ge(dve_ctr, n // 2 + 1)
                if i in (0, 1):
                    t.wait_ge(dve_qtr, b + 1)     # bank0 quarters held QT
                ap = n - 2 - (n % 2)              # exp of evicted/conflicting quarter done
                if ap >= 1:
                    t.wait_ge(act_p, ap)          # also implies dve_nm >= n//4 transitively
                elif n >= 4:
                    t.wait_ge(dve_nm, n // 4)
                mm0 = t.matmul(pS[:, q, :], mcs[b % 2][:, i * 128:(i + 1) * 128],
                               mqs[b % 2][:], start=True, stop=False)
                pp = (n // 2) % 2
                t.matmul(pS[:, q, :], ctr[pp][:, k, 0], qtr[b % 2][:, 0], start=False, stop=False)
                t.matmul(pS[:, q, :], ctr[pp][:, k, 1], qtr[b % 2][:, 1], start=False, stop=True).then_inc(pe_s, 1)

            def pt_tr(n):
                k = n % 2
                if n == 0:
                    t.wait_ge(s_misc, 1)    # ident_b ready
                if n >= 2:
                    t.wait_ge(dve_ptr, n // 2)   # pair copy of (n-2) done (whole bank)
                ptb = pPT[:].bitcast(BF16)
                tr0 = t.transpose(ptb[:, k * 256:k * 256 + 128], p_sb[n % 4][:, 0:128], ident_b[:])
                tr0._wait_ge(act_p, n + 1)
                t.transpose(ptb[:, k * 256 + 128:k * 256 + 256], p_sb[n % 4][:, 128:256],
                            ident_b[:]).then_inc(pe_pt, 1)

            def mm2(n):
                b, i = divmod(n, NBLK)
                k = n % 2
                if n >= 2:
                    t.wait_ge(act_o, n - 1)   # outcp(n-2) done (own bank)
                pp = (n // 2) % 2
                mm0 = t.matmul(pO[k][:], ptr[pp][:, k, 0], qn_b[b % 2][:, 0], start=True, stop=False)
                mm0._wait_ge(dve_ptr, n // 2 + 1)
                t.matmul(pO[k][:], ptr[pp][:, k, 1], qn_b[b % 2][:, 1], start=False, stop=True).then_inc(pe_o, 1)

            def qt_prep(b):
                t.wait_ge(s_cin, cin_thresh(b))
                if b >= 1:
                    t.wait_ge(dve_qtr, b)       # prev QT copy done
                    t.wait_ge(act_p, 16 * b)    # pS bank0 prior exps done
                    t.wait_ge(dve_nm, 4 * b)    # prior quad reads done
                psr = pS[:].bitcast(F32R)
                last = None
                for qa in range(2):
                    for kk in range(2):
                        last = t.transpose(
                            psr[:, kk, qa * 128:(qa + 1) * 128],
                            qn[b % 2][:, qa, kk * 128:(kk + 1) * 128],
                            ident[:],
                        )
                last.then_inc(pe_qt, 1)

            def tail(b):
                # C: q2c matmuls + total sum (constant-shift exp, no global max)
                t.wait_ge(dt, 2 * b + 1)      # esum_r ready
                t.wait_ge(at, 2 * b + 1)      # E_all ready
                if b >= 1:
                    t.wait_ge(at, 2 * b)      # T2(b-1) done reading pM
                for i in range(NBLK):
                    t.matmul(pM[0:1, 0:256], E_all[:, i:i + 1], cb[b % 2][:, i, :],
                             start=(i == 0), stop=(i == NBLK - 1))
                t.matmul(pM[0:1, 256:512], esum_r[:], ones_w[:], start=True,
                         stop=True).then_inc(pt_, 1)

            for b in range(NB):
                qt_prep(b)
                for slot in range(NBLK + 12):
                    i = slot - 6
                    if 0 <= i <= NBLK - 1:
                        pt_tr(16 * b + i)
                    i = slot - 8
                    if 0 <= i <= NBLK - 1:
                        mm2(16 * b + i)
                    i = slot
                    if 0 <= i <= NBLK - 1:
                        ct_tr(16 * b + i)
                    i = slot - 2
                    if 0 <= i <= NBLK - 1:
                        sim(16 * b + i)
                tail(b)

        # ---------------- DVE ----------------
        @blk.vector
        def _(v):
            def qtr_copy(b):
                if b == 0:
                    v.wait_ge(s_cin, cin_thresh(0))
                    v.tensor_copy(ident_b[:], ident[:]).then_inc(s_misc, 1)
                v.wait_ge(pe_qt, b + 1)
                if b >= 2:
                    v.wait_ge(pe_o, 16 * (b - 1))   # qn_b WAR (implies pe_s too)
                v.tensor_copy(qn_b[b % 2][:], qn[b % 2][:])
                v.tensor_copy(qtr[b % 2][:], pS[:].bitcast(F32R)[:, 0:2, :]).then_inc(dve_qtr, 1)

            def ctr_pair(b, p):
                # copy C^T for blocks 16b+2p, +2p+1 in one op
                n1 = 16 * b + 2 * p + 1
                if n1 >= 5:
                    v.wait_ge(pe_s, n1 - 3)       # sims of pair evicted 2 pairs ago done
                cp = v.tensor_copy(ctr[p % 2][:], pJ[:])
                cp._wait_ge(pe_ct, n1 + 1)
                cp.then_inc(dve_ctr, 1)

            def nm_quad(b, qq):
                # one reduce for blocks 16b+4qq .. +3
                i4 = 4 * qq
                if qq == 0 and b >= 2:
                    v.wait_ge(at, 2 * (b - 2) + 1)   # tail(b-2) E-exp read NM buffer
                rd = v.tensor_reduce(NM[b % 2][:, i4:i4 + 4], pS[:], AX.X, OP.max,
                                     negate=True)
                rd._wait_ge(pe_s, 16 * b + 4 * qq + 4)
                rd.then_inc(dve_nm, 1)

            def ptr_pair(b, p):
                n1 = 16 * b + 2 * p + 1
                if n1 >= 5:
                    v.wait_ge(pe_o, n1 - 3)       # mm2s of pair evicted 2 pairs ago done
                cp = v.tensor_copy(ptr[p % 2][:], pPT[:].bitcast(BF16)[:, 0:512])
                cp._wait_ge(pe_pt, n1 + 1)
                cp.then_inc(dve_ptr, 1)

            def recip(n):
                b, i = divmod(n, NBLK)
                if i == 0 and b >= 2:
                    v.wait_ge(act_o, 16 * (b - 1))   # RS WAR vs out-copy of b-2
                rc = v.reciprocal(RS[b % 2][:, i:i + 1], SS[b % 2][:, i:i + 1])
                rc._wait_ge(act_p, n + 1)
                rc.then_inc(dve_rs, 1)

            def tail(b):
                # X1: esum -> f32r
                v.wait_ge(at, 2 * b + 1)
                v.tensor_copy(esum_r[:], esum[:]).then_inc(dt, 1)
                # X2: total -> reciprocal
                v.wait_ge(pt_, b + 1)
                if b >= 1:
                    v.wait_ge(at, 2 * b)   # T2(b-1) done with rtot
                v.tensor_copy(t_sb[:], pM[0:1, 256:257])
                v.drain()
                v.reciprocal(rtot[:], t_sb[:]).then_inc(dt, 1)

            for b in range(NB):
                qtr_copy(b)
                for slot in range(NBLK + 12):
                    if slot >= 7 and slot % 2 == 1 and (slot - 7) // 2 <= 7:
                        ptr_pair(b, (slot - 7) // 2)
                    i = slot - 6
                    if 0 <= i <= NBLK - 1:
                        recip(16 * b + i)
                    if slot % 2 == 1 and (slot - 1) // 2 <= 7:
                        ctr_pair(b, (slot - 1) // 2)
                    if slot >= 5 and (slot - 5) % 4 == 0 and (slot - 5) // 4 <= 3:
                        nm_quad(b, (slot - 5) // 4)
                tail(b)

        # ---------------- ACT ----------------
        @blk.scalar
        def _(s):
            def ex(n):
                b, i = divmod(n, NBLK)
                q = n % 4
                if n >= 4:
                    s.wait_ge(pe_pt, n - 3)          # p_sb 4-deep WAR
                if i == 0 and b >= 2:
                    s.wait_ge(dve_rs, 16 * (b - 1))  # SS WAR vs recip of b-2
                ac = s.activation(p_sb[q][:], pS[:, q, :], Exp,
                                  bias=NM[b % 2][:, i:i + 1],
                                  accum_out=SS[b % 2][:, i:i + 1])
                ac._wait_ge(dve_nm, 4 * b + n % 16 // 4 + 1)
                ac.then_inc(act_p, 1)

            def outcp(n):
                b, i = divmod(n, NBLK)
                k = n % 2
                s.wait_ge(dve_rs, n + 1)
                if i == 0 and b >= 2:
                    s.wait_ge(s_out, 16 * (b - 1))
                oc = s.mul(o_all[b % 2][:, i, :], pO[k][:], RS[b % 2][:, i:i + 1])
                oc._wait_ge(pe_o, n + 1)
                oc.then_inc(act_o, 1)

            def tail(b):
                # T1: E = exp(-NM - 100), accum esum
                s.wait_ge(dve_nm, 4 * (b + 1))
                if b >= 1:
                    s.wait_ge(pt_, b)        # E_all/esum WAR vs tail C of b-1
                s.activation(E_all[:], NM[b % 2][:], Exp, bias=c100[:], scale=-1.0,
                             accum_out=esum[:]).then_inc(at, 1)
                # T2: qc = pQC * rtot
                s.wait_ge(dt, 2 * b + 2)
                s.wait_ge(pt_, b + 1)
                if b >= 2:
                    s.wait_ge(s_qc, 16 * (b - 1))
                s.mul(qc_sb[b % 2][:], pM[0:1, 0:256], rtot[:]).then_inc(at, 1)

            for b in range(NB):
                for slot in range(NBLK + 12):
                    i = slot - 9
                    if 0 <= i <= NBLK - 1:
                        outcp(16 * b + i)
                    i = slot - 4
                    if 0 <= i <= NBLK - 1:
                        ex(16 * b + i)
                tail(b)

        # ---------------- SYNC: output DMAs ----------------
        @blk.sync
        def _(sy):
            for b in range(NB):
                sy.wait_ge(act_o, 16 * (b + 1))
                if b >= 1:
                    sy.wait_ge(s_out, 16 * b)
                sy.dma_start(o_d[b].rearrange("(i p) d -> p i d", p=128),
                             o_all[b % 2][:]).then_inc(s_out, 16)
                sy.wait_ge(at, 2 * b + 2)
                if b >= 1:
                    sy.wait_ge(s_qc, 16 * b)
                sy.dma_start(qc_d[b:b + 1, :], qc_sb[b % 2][:]).then_inc(s_qc, 16)

    return nc, es


_CACHE = {}


def _get_program():
    if "nc" not in _CACHE:
        nc, es = build_program()
        _CACHE["nc"] = nc
        _CACHE["es"] = es
    return _CACHE["nc"]


def kernel(context_repr, question_repr, context_len, question_len):
    context_repr = np.ascontiguousarray(np.asarray(context_repr, np.float32))
    question_repr = np.ascontiguousarray(np.asarray(question_repr, np.float32))
    context_len = np.asarray(context_len, np.int32)
    question_len = np.asarray(question_len, np.int32)

    cm = (np.arange(TC)[None, :] < context_len[:, None]).astype(np.float32)  # [B,Tc]
    qm = (np.arange(TQ)[None, :] < question_len[:, None]).astype(np.float32)  # [B,Tq]
    mcf = np.stack([SQ * cm, np.ones_like(cm)], axis=1)                      # [B,2,Tc]
    mqf = np.stack([SQ * qm, np.full_like(qm, NEG)], axis=1)                 # [B,2,Tq]
    ident = np.eye(128, dtype=np.float32)
    onesw = np.ones((128, 256), np.float32)
    c100 = np.full((128, 1), -100.0, np.float32)

    nc = _get_program()
    in_maps = []
    for core in range(NCORES):
        sl = slice(core * NB, (core + 1) * NB)
        in_maps.append({
            "c": context_repr[sl],
            "q": question_repr[sl],
            "mcf": np.ascontiguousarray(mcf[sl]),
            "mqf": np.ascontiguousarray(mqf[sl]),
            "ident": ident,
            "onesw": onesw,
            "c100": c100,
        })

    res = run_bass_kernel_spmd(nc, in_maps, list(range(NCORES)))
    out1 = np.concatenate([np.asarray(r["o"]).reshape(NB, TC, D) for r in res.results], axis=0)
    q2c = np.concatenate([np.asarray(r["qc"]).reshape(NB, TQ) for r in res.results], axis=0)
    out2 = np.ascontiguousarray(np.broadcast_to(q2c[:, None, :], (B, TC, D)))
    return out1, out2


# revision 24
# speedup vs baseline: 1.3044x; 1.0051x over previous
"""BiAttention TRN2 kernel: data-parallel over batch across 8 NeuronCores.

Self-contained: hardcodes B=32, Tc=2048, Tq=256, D=256, 8 cores, 4 batches/core.
Raw-bass software-pipelined kernel; f32r matmuls; exact power-of-two mask trick.
"""
import numpy as np

import concourse.bass as bass
from concourse import mybir
from concourse.bass_utils import run_bass_kernel_spmd

F32 = mybir.dt.float32
F32R = mybir.dt.float32r
BF16 = mybir.dt.bfloat16
Exp = mybir.ActivationFunctionType.Exp
AX = mybir.AxisListType
OP = mybir.AluOpType

B, TC, TQ, D = 32, 2048, 256, 256
NCORES = 8
NB = B // NCORES          # batches per core = 4
NBLK = TC // 128          # c-blocks per batch = 16
NEG = -(2.0 ** 96)
SQ = 2.0 ** 48


def build_program():
    nc = bass.Bass()
    c_d = nc.declare_dram_parameter("c", [NB, TC, D], F32, isOutput=False)
    q_d = nc.declare_dram_parameter("q", [NB, TQ, D], F32, isOutput=False)
    mc_d = nc.declare_dram_parameter("mcf", [NB, 2, TC], F32, isOutput=False)
    mq_d = nc.declare_dram_parameter("mqf", [NB, 2, TQ], F32, isOutput=False)
    id_d = nc.declare_dram_parameter("ident", [128, 128], F32, isOutput=False)
    onew_d = nc.declare_dram_parameter("onesw", [128, 256], F32, isOutput=False)
    c100_d = nc.declare_dram_parameter("c100", [128, 1], F32, isOutput=False)

    o_d = nc.declare_dram_parameter("o", [NB, TC, D], F32, isOutput=True)
    qc_d = nc.declare_dram_parameter("qc", [NB, TQ], F32, isOutput=True)

    from contextlib import ExitStack
    es = ExitStack()
    _ctr = [0]

    def sb(shape, dt, name=None):
        _ctr[0] += 1
        return es.enter_context(nc.sbuf_tensor(name or f"sb{_ctr[0]}", shape, dt))

    def ps(shape, dt, name=None):
        _ctr[0] += 1
        return es.enter_context(nc.psum_tensor(name or f"ps{_ctr[0]}", shape, dt))

    def sem(name):
        return es.enter_context(nc.semaphore(name))

    # ---- SBUF ----
    cb = [sb([128, NBLK, D], F32R) for _ in range(2)]      # C natural (f32r), per-batch parity
    qn = [sb([128, 2, D], F32R) for _ in range(2)]          # Q natural [q%128, qchunk, d]
    qtr = [sb([128, 2, TQ], F32R) for _ in range(2)]        # Q^T [d%128, dchunk, q]
    mcs = [sb([2, TC], F32R) for _ in range(2)]             # mask lhsT features
    mqs = [sb([2, TQ], F32R) for _ in range(2)]             # mask rhs features
    ident = sb([128, 128], F32R)
    ones_w = sb([128, 256], F32R)                           # all-ones (total-sum rhs)
    c100 = sb([128, 1], F32)                                # bias constant -100
    ctr = [sb([128, 2, 2, 128], F32R) for _ in range(2)]    # C^T (par, chunk, c), pair-parity
    ptr = [sb([128, 2, 2, 128], BF16) for _ in range(2)]    # P^T (par, chunk, c), pair-parity
    p_sb = [sb([128, TQ], BF16) for _ in range(4)]          # exp(S-m) (bf16), 4-deep
    qn_b = [sb([128, 2, D], BF16) for _ in range(2)]        # Q natural bf16 (mm2 rhs)
    ident_b = sb([128, 128], BF16)
    o_all = [sb([128, NBLK, D], F32) for _ in range(2)]     # output batch buffer
    NM = [sb([128, NBLK], F32) for _ in range(2)]           # -rowmax per block column
    SS = [sb([128, NBLK], F32) for _ in range(2)]           # rowsum per block column
    RS = [sb([128, NBLK], F32) for _ in range(2)]           # 1/rowsum
    E_all = sb([128, NBLK], F32R)                           # exp(m - 100) for q2c
    esum = sb([128, 1], F32)
    esum_r = sb([128, 1], F32R)
    t_sb = sb([1, 1], F32)
    rtot = sb([1, 1], F32)
    qc_sb = [sb([1, TQ], F32) for _ in range(2)]

    # ---- PSUM (bank-granular allocator: 8 banks total) ----
    pJ = [ps([128, 2, 256], F32R) for _ in range(2)]  # C^T pair buffers (1 bank each)
    pPT = ps([128, 256], F32R)                      # P^T both parities (bf16 via bitcast), 1 bank
    pS = ps([128, 4, 256], F32)                     # sim quad (2 banks); QT prep borrows bank0 via f32r bitcast
    pO = [ps([128, 256], F32) for _ in range(2)]    # mm2 out, 1 bank each
    # pM regions: pQC=[0:1,0:256], pTot=[0:1,256:512]
    pM = ps([128, 512], F32)

    sems = {}
    for name in ("s_cin", "s_out", "s_qc", "pe_ct", "pe_qt", "pe_s", "pe_pt",
                 "pe_o", "pt_", "dve_ctr", "dve_qtr", "dve_nm", "dve_rs",
                 "dve_ptr", "dt", "act_p", "act_o", "at", "s_misc"):
        sems[name] = sem(name)
    s_cin = sems["s_cin"]; s_out = sems["s_out"]; s_qc = sems["s_qc"]
    pe_ct = sems["pe_ct"]; pe_qt = sems["pe_qt"]; pe_s = sems["pe_s"]
    pe_pt = sems["pe_pt"]; pe_o = sems["pe_o"]; pt_ = sems["pt_"]
    dve_ctr = sems["dve_ctr"]; dve_qtr = sems["dve_qtr"]; dve_nm = sems["dve_nm"]
    dve_rs = sems["dve_rs"]; dve_ptr = sems["dve_ptr"]; dt = sems["dt"]
    act_p = sems["act_p"]; act_o = sems["act_o"]; at = sems["at"]
    s_misc = sems["s_misc"]

    blk = es.enter_context(nc.Block())
    with blk:
        # ---------------- GPSIMD: input cast-DMAs ----------------
        @blk.gpsimd
        def _(g):
            for b in range(NB):
                if b >= 2:
                    g.wait_ge(pt_, b - 1)
                if b >= 1:
                    # all previously issued input DMAs must have completed so
                    # cumulative thresholds are meaningful (unordered DMA completion)
                    g.wait_ge(s_cin, 64 * b + 48)
                g.dma_start(cb[b % 2][:], c_d[b].rearrange("(i p) d -> p i d", p=128)).then_inc(s_cin, 16)
                g.dma_start(qn[b % 2][:], q_d[b].rearrange("(a p) d -> p a d", p=128)).then_inc(s_cin, 16)
                g.dma_start(mcs[b % 2][:], mc_d[b]).then_inc(s_cin, 16)
                g.dma_start(mqs[b % 2][:], mq_d[b]).then_inc(s_cin, 16)
                if b == 0:
                    g.dma_start(ident[:], id_d[:]).then_inc(s_cin, 16)
                    g.dma_start(ones_w[:], onew_d[:]).then_inc(s_cin, 16)
                    g.dma_start(c100[:], c100_d[:]).then_inc(s_cin, 16)

        def cin_thresh(b):
            return 64 * (b + 1) + 48

        # ---------------- PE ----------------
        @blk.tensor
        def _(t):
            def ct_tr(n):
                b, i = divmod(n, NBLK)
                k = n % 2
                if i == 0:
                    t.wait_ge(s_cin, cin_thresh(b))
                pp = (n // 2) % 2
                tr0 = t.transpose(pJ[pp][:, k, 0:128], cb[b % 2][:, i, 0:128], ident[:])
                if n >= 4:
                    tr0._wait_ge(dve_ctr, n // 2 - 1)   # pair copy 2 pairs back done
                t.transpose(pJ[pp][:, k, 128:256], cb[b % 2][:, i, 128:256], ident[:]).then_inc(pe_ct, 1)

            def sim(n):
                b, i = divmod(n, NBLK)
                k = n % 2
                q = n % 4
                t.wait_ge(dve_ctr, n // 2 + 1)
                if i in (0, 1):
                    t.wait_ge(dve_qtr, b + 1)     # bank0 quarters held QT
                ap = n - 2 - (n % 2)              # exp of evicted/conflicting quarter done
                if ap >= 1:
                    t.wait_ge(act_p, ap)          # also implies dve_nm >= n//4 transitively
                elif n >= 4:
                    t.wait_ge(dve_nm, n // 4)
                mm0 = t.matmul(pS[:, q, :], mcs[b % 2][:, i * 128:(i + 1) * 128],
                               mqs[b % 2][:], start=True, stop=False)
                pp = (n // 2) % 2
                t.matmul(pS[:, q, :], ctr[pp][:, k, 0], qtr[b % 2][:, 0], start=False, stop=False)
                t.matmul(pS[:, q, :], ctr[pp][:, k, 1], qtr[b % 2][:, 1], start=False, stop=True).then_inc(pe_s, 1)

            def pt_tr(n):
                k = n % 2
                if n == 0:
                    t.wait_ge(s_misc, 1)    # ident_b ready
                if n >= 2:
                    t.wait_ge(dve_ptr, n // 2)   # pair copy of (n-2) done (whole bank)
                ptb = pPT[:].bitcast(BF16)
                tr0 = t.transpose(ptb[:, k * 256:k * 256 + 128], p_sb[n % 4][:, 0:128], ident_b[:])
                tr0._wait_ge(act_p, n + 1)
                t.transpose(ptb[:, k * 256 + 128:k * 256 + 256], p_sb[n % 4][:, 128:256],
                            ident_b[:]).then_inc(pe_pt, 1)

            def mm2(n):
                b, i = divmod(n, NBLK)
                k = n % 2
                if n >= 2:
                    t.wait_ge(act_o, n - 1)   # outcp(n-2) done (own bank)
                pp = (n // 2) % 2
                mm0 = t.matmul(pO[k][:], ptr[pp][:, k, 0], qn_b[b % 2][:, 0], start=True, stop=False)
                mm0._wait_ge(dve_ptr, n // 2 + 1)
                t.matmul(pO[k][:], ptr[pp][:, k, 1], qn_b[b % 2][:, 1], start=False, stop=True).then_inc(pe_o, 1)

            def qt_prep(b):
                t.wait_ge(s_cin, cin_thresh(b))
                if b >= 1:
                    t.wait_ge(dve_qtr, b)       # prev QT copy done
                    t.wait_ge(act_p, 16 * b)    # pS bank0 prior exps done
                    t.wait_ge(dve_nm, 4 * b)    # prior quad reads done
                psr = pS[:].bitcast(F32R)
                last = None
                for qa in range(2):
                    for kk in range(2):
                        last = t.transpose(
                            psr[:, kk, qa * 128:(qa + 1) * 128],
                            qn[b % 2][:, qa, kk * 128:(kk + 1) * 128],
                            ident[:],
                        )
                last.then_inc(pe_qt, 1)

            def tail(b):
                # C: q2c matmuls + total sum (constant-shift exp, no global max)
                t.wait_ge(dt, 2 * b + 1)      # esum_r ready
                t.wait_ge(at, 2 * b + 1)      # E_all ready
                if b >= 1:
                    t.wait_ge(at, 2 * b)      # T2(b-1) done reading pM
                for i in range(NBLK):
                    t.matmul(pM[0:1, 0:256], E_all[:, i:i + 1], cb[b % 2][:, i, :],
                             start=(i == 0), stop=(i == NBLK - 1))
                t.matmul(pM[0:1, 256:512], esum_r[:], ones_w[:], start=True,
                         stop=True).then_inc(pt_, 1)

            for b in range(NB):
                qt_prep(b)
                for slot in range(NBLK + 12):
                    i = slot - 6
                    if 0 <= i <= NBLK - 1:
                        pt_tr(16 * b + i)
                    i = slot - 8
                    if 0 <= i <= NBLK - 1:
                        mm2(16 * b + i)
                    i = slot
                    if 0 <= i <= NBLK - 1:
                        ct_tr(16 * b + i)
                    i = slot - 2
                    if 0 <= i <= NBLK - 1:
                        sim(16 * b + i)
                tail(b)

        # ---------------- DVE ----------------
        @blk.vector
        def _(v):
            def qtr_copy(b):
                if b == 0:
                    v.wait_ge(s_cin, cin_thresh(0))
                    v.tensor_copy(ident_b[:], ident[:]).then_inc(s_misc, 1)
                v.wait_ge(pe_qt, b + 1)
                if b >= 2:
                    v.wait_ge(pe_o, 16 * (b - 1))   # qn_b WAR (implies pe_s too)
                v.tensor_copy(qn_b[b % 2][:], qn[b % 2][:])
                v.tensor_copy(qtr[b % 2][:], pS[:].bitcast(F32R)[:, 0:2, :]).then_inc(dve_qtr, 1)

            def ctr_pair(b, p):
                # copy C^T for blocks 16b+2p, +2p+1 in one op
                n1 = 16 * b + 2 * p + 1
                if n1 >= 5:
                    v.wait_ge(pe_s, n1 - 3)       # sims of pair evicted 2 pairs ago done
                cp = v.tensor_copy(ctr[p % 2][:], pJ[p % 2][:])
                cp._wait_ge(pe_ct, n1 + 1)
                cp.then_inc(dve_ctr, 1)

            def nm_quad(b, qq):
                # one reduce for blocks 16b+4qq .. +3
                i4 = 4 * qq
                if qq == 0 and b >= 2:
                    v.wait_ge(at, 2 * (b - 2) + 1)   # tail(b-2) E-exp read NM buffer
                rd = v.tensor_reduce(NM[b % 2][:, i4:i4 + 4], pS[:], AX.X, OP.max,
                                     negate=True)
                rd._wait_ge(pe_s, 16 * b + 4 * qq + 4)
                rd.then_inc(dve_nm, 1)

            def ptr_pair(b, p):
                n1 = 16 * b + 2 * p + 1
                if n1 >= 5:
                    v.wait_ge(pe_o, n1 - 3)       # mm2s of pair evicted 2 pairs ago done
                cp = v.tensor_copy(ptr[p % 2][:], pPT[:].bitcast(BF16)[:, 0:512])
                cp._wait_ge(pe_pt, n1 + 1)
                cp.then_inc(dve_ptr, 1)

            def recip(n):
                b, i = divmod(n, NBLK)
                if i == 0 and b >= 2:
                    v.wait_ge(act_o, 16 * (b - 1))   # RS WAR vs out-copy of b-2
                rc = v.reciprocal(RS[b % 2][:, i:i + 1], SS[b % 2][:, i:i + 1])
                rc._wait_ge(act_p, n + 1)
                rc.then_inc(dve_rs, 1)

            def tail(b):
                # X1: esum -> f32r
                v.wait_ge(at, 2 * b + 1)
                v.tensor_copy(esum_r[:], esum[:]).then_inc(dt, 1)
                # X2: total -> reciprocal
                v.wait_ge(pt_, b + 1)
                if b >= 1:
                    v.wait_ge(at, 2 * b)   # T2(b-1) done with rtot
                v.tensor_copy(t_sb[:], pM[0:1, 256:257])
                v.drain()
                v.reciprocal(rtot[:], t_sb[:]).then_inc(dt, 1)

            for b in range(NB):
                qtr_copy(b)
                for slot in range(NBLK + 12):
                    if slot >= 7 and slot % 2 == 1 and (slot - 7) // 2 <= 7:
                        ptr_pair(b, (slot - 7) // 2)
                    i = slot - 6
                    if 0 <= i <= NBLK - 1:
                        recip(16 * b + i)
                    if slot % 2 == 1 and (slot - 1) // 2 <= 7:
                        ctr_pair(b, (slot - 1) // 2)
                    if slot >= 5 and (slot - 5) % 4 == 0 and (slot - 5) // 4 <= 3:
                        nm_quad(b, (slot - 5) // 4)
                tail(b)

        # ---------------- ACT ----------------
        @blk.scalar
        def _(s):
            def ex(n):
                b, i = divmod(n, NBLK)
                q = n % 4
                if n >= 4:
                    s.wait_ge(pe_pt, n - 3)          # p_sb 4-deep WAR
                if i == 0 and b >= 2:
                    s.wait_ge(dve_rs, 16 * (b - 1))  # SS WAR vs recip of b-2
                ac = s.activation(p_sb[q][:], pS[:, q, :], Exp,
                                  bias=NM[b % 2][:, i:i + 1],
                                  accum_out=SS[b % 2][:, i:i + 1])
                ac._wait_ge(dve_nm, 4 * b + n % 16 // 4 + 1)
                ac.then_inc(act_p, 1)

            def outcp(n):
                b, i = divmod(n, NBLK)
                k = n % 2
                s.wait_ge(dve_rs, n + 1)
                if i == 0 and b >= 2:
                    s.wait_ge(s_out, 16 * (b - 1))
                oc = s.mul(o_all[b % 2][:, i, :], pO[k][:], RS[b % 2][:, i:i + 1])
                oc._wait_ge(pe_o, n + 1)
                oc.then_inc(act_o, 1)

            def tail(b):
                # T1: E = exp(-NM - 100), accum esum
                s.wait_ge(dve_nm, 4 * (b + 1))
                if b >= 1:
                    s.wait_ge(pt_, b)        # E_all/esum WAR vs tail C of b-1
                s.activation(E_all[:], NM[b % 2][:], Exp, bias=c100[:], scale=-1.0,
                             accum_out=esum[:]).then_inc(at, 1)
                # T2: qc = pQC * rtot
                s.wait_ge(dt, 2 * b + 2)
                s.wait_ge(pt_, b + 1)
                if b >= 2:
                    s.wait_ge(s_qc, 16 * (b - 1))
                s.mul(qc_sb[b % 2][:], pM[0:1, 0:256], rtot[:]).then_inc(at, 1)

            for b in range(NB):
                for slot in range(NBLK + 12):
                    i = slot - 9
                    if 0 <= i <= NBLK - 1:
                        outcp(16 * b + i)
                    i = slot - 4
                    if 0 <= i <= NBLK - 1:
                        ex(16 * b + i)
                tail(b)

        # ---------------- SYNC: output DMAs ----------------
        @blk.sync
        def _(sy):
            for b in range(NB):
                sy.wait_ge(act_o, 16 * (b + 1))
                if b >= 1:
                    sy.wait_ge(s_out, 16 * b)
                sy.dma_start(o_d[b].rearrange("(i p) d -> p i d", p=128),
                             o_all[b % 2][:]).then_inc(s_out, 16)
                sy.wait_ge(at, 2 * b + 2)
                if b >= 1:
                    sy.wait_ge(s_qc, 16 * b)
                sy.dma_start(qc_d[b:b + 1, :], qc_sb[b % 2][:]).then_inc(s_qc, 16)

    return nc, es


_CACHE = {}


def _get_program():
    if "nc" not in _CACHE:
        nc, es = build_program()
        _CACHE["nc"] = nc
        _CACHE["es"] = es
    return _CACHE["nc"]


def kernel(context_repr, question_repr, context_len, question_len):
    context_repr = np.ascontiguousarray(np.asarray(context_repr, np.float32))
    question_repr = np.ascontiguousarray(np.asarray(question_repr, np.float32))
    context_len = np.asarray(context_len, np.int32)
    question_len = np.asarray(question_len, np.int32)

    cm = (np.arange(TC)[None, :] < context_len[:, None]).astype(np.float32)  # [B,Tc]
    qm = (np.arange(TQ)[None, :] < question_len[:, None]).astype(np.float32)  # [B,Tq]
    mcf = np.stack([SQ * cm, np.ones_like(cm)], axis=1)                      # [B,2,Tc]
    mqf = np.stack([SQ * qm, np.full_like(qm, NEG)], axis=1)                 # [B,2,Tq]
    ident = np.eye(128, dtype=np.float32)
    onesw = np.ones((128, 256), np.float32)
    c100 = np.full((128, 1), -100.0, np.float32)

    nc = _get_program()
    in_maps = []
    for core in range(NCORES):
        sl = slice(core * NB, (core + 1) * NB)
        in_maps.append({
            "c": context_repr[sl],
            "q": question_repr[sl],
            "mcf": np.ascontiguousarray(mcf[sl]),
            "mqf": np.ascontiguousarray(mqf[sl]),
            "ident": ident,
            "onesw": onesw,
            "c100": c100,
        })

    res = run_bass_kernel_spmd(nc, in_maps, list(range(NCORES)))
    out1 = np.concatenate([np.asarray(r["o"]).reshape(NB, TC, D) for r in res.results], axis=0)
    q2c = np.concatenate([np.asarray(r["qc"]).reshape(NB, TQ) for r in res.results], axis=0)
    out2 = np.ascontiguousarray(np.broadcast_to(q2c[:, None, :], (B, TC, D)))
    return out1, out2
